# revision 39
# baseline (speedup 1.0000x reference)
"""Trainium2 Bass kernel for nn_AttnReadout (segment attention readout).

Computation (reference):
    anchor[b]  = mean of ifeat rows in segment b                  [B, D]
    e[i]       = sigmoid(ifeat @ Wu.T + (anchor @ Wv.T + bv)[seg]) @ we
    alpha      = segment_softmax(e)
    rst[b]     = sum_i alpha[i] * ifeat[i]                        [B, D]
    out        = concat([rst, anchor], axis=1)                    [B, 2D]

Sharding: 2048 segments -> 8 cores x 2 windows of 128 contiguous segments.
Nodes (sorted by segment) are padded per-window to T_W tiles of 128 rows.

All heavy matmuls run in fp8e4 with MatmulPerfMode.DoubleRow (2 k-tiles per
instruction, 0.5 cycles per output column):
  - anchor / weighted-sum segment reductions pair ADJACENT node tiles as the
    two k-tiles (one-hot pair as stationary, feature pair as moving).
  - fc_u contracts the two 128-feature halves as k-tiles.
  - the fv gather duplicates its single 128-seg k-tile via stride-0 APs and
    pre-halves fv to compensate.
One-hot matrices in both layouts (ohw [node,seg] and ohT [seg,node]) are
host-prepared from seg_ids and DMAed as fp8 (no on-device transposes).
Sigmoid runs on batches of 4 node tiles from a 2-bank PSUM region to
amortize the activation-engine access latency; z = exp(e) uses the
sigmoid(e)/sigmoid(-e) identity to stay on one ACT table.

Scaling: Wu.T and Wv.T are stored x16 (good fp8 range), anchor means x4,
fv stored as 8*(fv+bv) (halved for the stride-0 duplication), and the
sigmoid applies scale=1/16 to undo it. The z scale cancels in rst/denom.
"""

import numpy as np
import ml_dtypes

N = 102400
D = 256
B = 2048
N_CORES = 8
W_PER_CORE = 2
N_WINDOWS = N_CORES * W_PER_CORE  # 16
SEGS_PER_WINDOW = B // N_WINDOWS  # 128
P = 128
BF = ml_dtypes.bfloat16
F8 = ml_dtypes.float8_e4m3fn

WU_SCALE = 16.0
ANCH_SCALE = 4.0
FV_SCALE = 8.0  # fv stored as FV_SCALE*(fv+bv); doubled by dup -> 16
SB = 4          # sigmoid batch (node tiles per activation)


def _apply_tile_patch():
    """Split TileContext's multi-wait tail drain into single-wait drains
    (this walrus build rejects >1 sync wait on a Drain instruction)."""
    import concourse.tile as tile_mod
    from concourse.vector_clock import ScopedClock

    if getattr(tile_mod.TileContext, "_drain_wait_split_patch", False):
        return

    def _patched(self, tick_clock, wait_clock):
        nc = self.nc
        drain_inst = nc.sync.drain()
        wait_clock.add_sem_waits(
            drain_inst.ins, ScopedClock({None: tick_clock.global_clock})
        )
        si = drain_inst.ins.sync_info
        waits = list(si.on_wait) if si is not None else []
        if len(waits) > 1:
            SyncInfo = type(si)
            drain_inst.ins.sync_info = SyncInfo(
                on_wait=[waits[0]], on_update=list(si.on_update)
            )
            for w in waits[1:]:
                extra = nc.sync.drain()
                extra.ins.sync_info = SyncInfo(on_wait=[w], on_update=[])

        nc.all_engine_barrier()
        assert self.sems is not None
        popped = nc._tile_sem_poison_stack.pop()
        assert popped is self._sem_poison
        nc.clear_and_free_semaphores(list(self.sems.allocated().values()))
        nc.all_engine_barrier()

    tile_mod.TileContext._drain_and_barrier = _patched
    tile_mod.TileContext._drain_wait_split_patch = True


def _split_sync_waits(nc, limit=1):
    """Split >limit sync waits per instruction into preceding single-wait
    EventSemaphore carriers on the same engine (walrus build limit)."""
    import concourse.mybir as mybir

    n_new = 0
    for _, bassbb in nc.bb_map.items():
        insts = bassbb.bb.instructions  # live list
        snapshot = list(insts)
        offset = 0
        for pos, inst in enumerate(snapshot):
            si = getattr(inst, "sync_info", None)
            if si is None:
                continue
            waits = list(si.on_wait)
            if len(waits) <= limit:
                continue
            SyncInfo = type(si)
            inst.sync_info = SyncInfo(
                on_wait=waits[:limit], on_update=list(si.on_update))
            carriers = []
            for w in waits[limit:]:
                c = mybir.InstEventSemaphore(
                    name=f"WSPLIT-{nc.next_id()}", ins=[], outs=[])
                c.engine = inst.engine
                c.sync_info = SyncInfo(on_wait=[w], on_update=[])
                carriers.append(c)
            insts[pos + offset:pos + offset] = carriers
            offset += len(carriers)
            n_new += len(carriers)
    return n_new


_PACK = {}


def _build(T_W, repeat=1, loop_repeat=None):
    """Build the single-core SPMD Bass program; T_W must be a multiple of 4.

    Uses _PACK["bases"] (canonical per-tile 32-aligned segment-block base,
    identical across cores) and _PACK["width"] set by _prepare: the one-hot
    of tile t only has nonzero columns in [base(t), base(t)+width)."""
    import contextlib
    import concourse.bass as bass
    import concourse.mybir as mybir
    from concourse.tile import TileContext

    _apply_tile_patch()

    f32 = mybir.dt.float32
    bf16 = mybir.dt.bfloat16
    fp8 = mybir.dt.float8e4
    Alu = mybir.AluOpType
    Act = mybir.ActivationFunctionType
    PM = mybir.MatmulPerfMode

    assert T_W % 4 == 0
    CH = T_W // 2            # tiles per DMA chunk (2 chunks per window), even
    NT = W_PER_CORE * T_W
    NB = T_W // SB           # sigmoid batches per window
    NB0 = NB // 2            # batches in z-chunk 0
    TC0 = NB0 * SB           # tiles in z-chunk 0
    NP = T_W // 2            # node-tile pairs per window
    NP0 = TC0 // 2           # pairs fully covered by z-chunk 0

    nc = bass.Bass("TRN2", num_devices=N_CORES)

    nat_dram = nc.dram_tensor("natp", [P, NT, D], fp8, kind="ExternalInput")
    ifT_dram = nc.dram_tensor("iftp", [P, NT, 2, P], fp8, kind="ExternalInput")
    ohw_dram = nc.dram_tensor("ohwp", [P, NT, P], fp8, kind="ExternalInput")
    ohT_dram = nc.dram_tensor("ohtp", [P, NT, P], fp8, kind="ExternalInput")
    wuT_dram = nc.dram_tensor("wuT8", [P, 2, D], fp8, kind="ExternalInput")
    wvT_dram = nc.dram_tensor("wvT8", [P, 2, D], fp8, kind="ExternalInput")
    web_dram = nc.dram_tensor("web", [P, 2], bf16, kind="ExternalInput")
    bvb_dram = nc.dram_tensor("bvb8", [P, D], f32, kind="ExternalInput")
    idb_dram = nc.dram_tensor("idb", [P, P], bf16, kind="ExternalInput")
    one_dram = nc.dram_tensor("one8", [P, 2], fp8, kind="ExternalInput")
    rcnt_dram = nc.dram_tensor("rcnt", [W_PER_CORE, P, 1], f32,
                               kind="ExternalInput")
    out_dram = nc.dram_tensor("out", [W_PER_CORE, P, 2 * D], f32,
                              kind="ExternalOutput")
    dbg_dram = None
    if _DEBUG:
        dbg_dram = nc.dram_tensor("dbg", [W_PER_CORE, P, 2 * T_W], f32,
                                  kind="ExternalOutput")

    with TileContext(nc) as tc:
        with contextlib.ExitStack() as ctx:
            const_pool = ctx.enter_context(tc.tile_pool(name="const", bufs=1))
            data_pool = ctx.enter_context(tc.tile_pool(name="data", bufs=1))
            sbuf_pool = ctx.enter_context(tc.tile_pool(name="sbuf", bufs=1))
            # PSUM: s_ps 2x2 banks + anchor 1 (tile also hosts the e columns
            # and transpose scratch) + wsum 1 + fv/wlo 2 (cross-rep overlap:
            # next rep's fv must not wait on this rep's final output) = 8.
            anchor_ps_pool = ctx.enter_context(
                tc.tile_pool(name="anchor_ps", bufs=1, space="PSUM"))
            wsum_ps_pool = ctx.enter_context(
                tc.tile_pool(name="wsum_ps", bufs=1, space="PSUM"))
            s_ps_pool = ctx.enter_context(
                tc.tile_pool(name="s_ps", bufs=2, space="PSUM"))
            fv_ps_pool = ctx.enter_context(
                tc.tile_pool(name="fv_ps", bufs=2, space="PSUM"))

            # ---- constants ----
            wuT_sb = const_pool.tile([P, 2, D], fp8, name="wuT_sb", tag="wuT_sb")
            nc.sync.dma_start(wuT_sb[:], wuT_dram[:])
            wvT_sb = const_pool.tile([P, 2, D], fp8, name="wvT_sb", tag="wvT_sb")
            nc.sync.dma_start(wvT_sb[:], wvT_dram[:])
            web_sb = const_pool.tile([P, 2], bf16, name="web_sb", tag="web_sb")
            nc.sync.dma_start(web_sb[:], web_dram[:])
            bvb_sb = const_pool.tile([P, D], f32, name="bvb_sb", tag="bvb_sb")
            nc.sync.dma_start(bvb_sb[:], bvb_dram[:])
            idb_sb = const_pool.tile([P, P], bf16, name="idb_sb", tag="idb_sb")
            nc.sync.dma_start(idb_sb[:], idb_dram[:])
            one_sb = const_pool.tile([P, 2], fp8, name="one_sb", tag="one_sb")
            nc.sync.dma_start(one_sb[:], one_dram[:])
            rcnt_sb = const_pool.tile([P, W_PER_CORE, 1], f32, name="rcnt_sb",
                                      tag="rcnt_sb")
            nc.sync.dma_start(
                rcnt_sb[:], rcnt_dram[:].rearrange("w p one -> p w one"))

            def body(rep):
                # window data, 2 chunks per window, all resident.
                # DMA emission follows consumption order: the anchor pass of a
                # window needs ohw+nat of both its chunks first; ifT/ohT feed
                # the later s-pipeline.
                nat_ch, ifT_ch, ohw_ch, ohT_ch = {}, {}, {}, {}

                def dma_ohw_nat(c):
                    ohwc = data_pool.tile([P, CH, P], fp8,
                                          name=f"ohwc{rep}_{c}", tag="ohwc",
                                          bufs=4)
                    nc.sync.dma_start(ohwc[:], ohw_dram[:, c * CH:(c + 1) * CH, :])
                    ohw_ch[c] = ohwc
                    natc = data_pool.tile([P, CH, D], fp8,
                                          name=f"natc{rep}_{c}", tag="natc",
                                          bufs=4)
                    nc.sync.dma_start(natc[:], nat_dram[:, c * CH:(c + 1) * CH, :])
                    nat_ch[c] = natc

                def dma_ift_oht(c):
                    iftc = data_pool.tile([P, CH, 2, P], fp8,
                                          name=f"iftc{rep}_{c}", tag="iftc",
                                          bufs=4)
                    nc.sync.dma_start(iftc[:], ifT_dram[:, c * CH:(c + 1) * CH, :, :])
                    ifT_ch[c] = iftc
                    ohtc = data_pool.tile([P, CH, P], fp8,
                                          name=f"ohtc{rep}_{c}", tag="ohtc",
                                          bufs=4)
                    nc.sync.dma_start(ohtc[:], ohT_dram[:, c * CH:(c + 1) * CH, :])
                    ohT_ch[c] = ohtc

                dma_ohw_nat(0)
                dma_ohw_nat(1)
                dma_ift_oht(0)
                dma_ift_oht(1)
                dma_ohw_nat(2)
                dma_ohw_nat(3)
                dma_ift_oht(2)
                dma_ift_oht(3)

                def nat_pair(w, j):      # [P, 2, D] node-tile pair
                    g = w * T_W + 2 * j
                    return nat_ch[g // CH][:, (g % CH):(g % CH) + 2, :]

                def ohw_pair(w, j):      # [P, 2, P]
                    g = w * T_W + 2 * j
                    return ohw_ch[g // CH][:, (g % CH):(g % CH) + 2, :]

                def ifT_t(w, t):         # [P, 2, P]
                    g = w * T_W + t
                    return ifT_ch[g // CH][:, g % CH, :, :]

                def ohT_t(w, t):         # [P, P]
                    g = w * T_W + t
                    return ohT_ch[g // CH][:, g % CH, :]

                def ohw_t(w, t):         # [P, P]
                    g = w * T_W + t
                    return ohw_ch[g // CH][:, g % CH, :]

                # per-window state
                st = {}

                def emit_anchor_pair(w, j):
                    nc.tensor.matmul(st[w]["anchor_ps"][:, 0:D], ohw_pair(w, j),
                                     nat_pair(w, j), start=(j == 0),
                                     stop=(j == NP - 1), perf_mode=PM.DoubleRow)

                def emit_window_setup(w):
                    # anchor bank also hosts the per-batch e columns [D:D+T_W]
                    # and the two bf16 transpose scratch regions after them
                    anchor_ps = anchor_ps_pool.tile(
                        [P, D + T_W + P], f32, name=f"anc{rep}_{w}",
                        tag="anchor_ps")
                    out_sb = sbuf_pool.tile([P, 2 * D], f32, name=f"osb{rep}_{w}",
                                            tag="out_sb", bufs=2)
                    e_win = sbuf_pool.tile([P, T_W], f32, name=f"ew{rep}_{w}",
                                           tag="e_win", bufs=2)
                    z_win = sbuf_pool.tile([P, T_W], f32, name=f"zw{rep}_{w}",
                                           tag="z_win", bufs=2)
                    z8_win = sbuf_pool.tile([P, T_W], f32, name=f"z8{rep}_{w}",
                                            tag="z8_win", bufs=2)
                    zlo_win = sbuf_pool.tile([P, T_W], f32, name=f"zl{rep}_{w}",
                                             tag="zlo_win", bufs=2)
                    wsum_ps = wsum_ps_pool.tile([P, D + 1], f32,
                                                name=f"ws{rep}_{w}", tag="wsum_ps")
                    st[w] = dict(anchor_ps=anchor_ps, out_sb=out_sb,
                                 e_win=e_win, z_win=z_win, z8_win=z8_win,
                                 zlo_win=zlo_win, wsum_ps=wsum_ps)

                def emit_fv(w):
                    anchor_ps = st[w]["anchor_ps"]
                    rc = rcnt_sb[:, w, :]
                    # anchor output half (Act engine keeps DVE free; reads PSUM)
                    nc.scalar.mul(st[w]["out_sb"][:, D:2 * D], anchor_ps[:, 0:D],
                                  rc)
                    anchb = sbuf_pool.tile([P, D], bf16, name=f"a8{rep}_{w}",
                                           tag="anchb", bufs=1)
                    nc.vector.tensor_scalar(anchb[:], anchor_ps[:, 0:D], rc,
                                            ANCH_SCALE, Alu.mult, Alu.mult)
                    anchT = sbuf_pool.tile([P, 2, P], fp8, name=f"aT{rep}_{w}",
                                           tag="anchT", bufs=1)
                    trbase = D + T_W
                    for db in range(2):
                        trp = (anchor_ps[:, trbase + db * (P // 2):
                                         trbase + (db + 1) * (P // 2)]
                               .bitcast(bf16))
                        nc.tensor.transpose(trp, anchb[:, db * P:(db + 1) * P],
                                            idb_sb[:])
                        nc.scalar.copy(anchT[:, db, :], trp)
                    fv_ps = fv_ps_pool.tile([P, D], f32, name=f"fvp{rep}_{w}",
                                            tag="fv_ps")
                    nc.tensor.matmul(fv_ps[:], anchT[:], wvT_sb[:],
                                     start=True, stop=True, perf_mode=PM.DoubleRow)
                    fv8 = sbuf_pool.tile([P, D], fp8, name=f"fv8{rep}_{w}",
                                         tag="fv8", bufs=1)
                    # fv8 = fv_ps * (FV_SCALE / (WU_SCALE*ANCH_SCALE)) + bv*FV_SCALE
                    nc.vector.scalar_tensor_tensor(
                        out=fv8[:], in0=fv_ps[:],
                        scalar=FV_SCALE / (WU_SCALE * ANCH_SCALE), in1=bvb_sb[:],
                        op0=Alu.mult, op1=Alu.add)
                    st[w]["fv8"] = fv8

                def emit_s_matmuls(w, b):
                    # transposed s: sT[do_half, node] per (half, q); the e
                    # reduction is then a ~free stationary-sT matmul with the
                    # we column as the 1-wide moving operand.
                    s_ps = s_ps_pool.tile([P, 2, SB, P], f32,
                                          name=f"sp{rep}_{w}_{b}", tag="s_ps")
                    fv8 = st[w]["fv8"]
                    for q in range(SB):
                        t = b * SB + q
                        oht_dup = ohT_t(w, t).unsqueeze(1).broadcast_to([P, 2, P])
                        for h in range(2):
                            nc.tensor.matmul(s_ps[:, h, q, :],
                                             wuT_sb[:, :, h * P:(h + 1) * P],
                                             ifT_t(w, t), start=True, stop=False,
                                             perf_mode=PM.DoubleRow)
                            fv_dup = (fv8[:, h * P:(h + 1) * P]
                                      .unsqueeze(1).broadcast_to([P, 2, P]))
                            nc.tensor.matmul(s_ps[:, h, q, :], fv_dup, oht_dup,
                                             start=False, stop=True,
                                             perf_mode=PM.DoubleRow)
                    s_sb = sbuf_pool.tile([P, 2, SB, P], bf16,
                                          name=f"ss{rep}_{w}_{b}", tag="s_sb",
                                          bufs=3)
                    nc.scalar.activation(
                        s_sb[:].rearrange("p h q d -> p (h q d)"),
                        s_ps[:].rearrange("p h q d -> p (h q d)"),
                        Act.Sigmoid, scale=1.0 / WU_SCALE)
                    st[w][f"s_sb{b}"] = s_sb

                def emit_e_batch(w, b):
                    anchor_ps = st[w]["anchor_ps"]
                    s_sb = st[w].pop(f"s_sb{b}")
                    for q in range(SB):
                        ec = D + b * SB + q
                        for h in range(2):
                            nc.tensor.matmul(anchor_ps[:, ec:ec + 1],
                                             s_sb[:, h, q, :],
                                             web_sb[:, h:h + 1],
                                             start=(h == 0), stop=(h == 1))
                    nc.vector.tensor_copy(
                        st[w]["e_win"][:, b * SB:(b + 1) * SB],
                        anchor_ps[:, D + b * SB:D + (b + 1) * SB])

                def emit_z_chunk(w, c0, c1):
                    e_win, z_win = st[w]["e_win"], st[w]["z_win"]
                    n = c1 - c0
                    sp = sbuf_pool.tile([P, n], f32, name=f"zp{rep}_{w}_{c0}",
                                        tag="zch", bufs=4)
                    nc.scalar.activation(sp[:], e_win[:, c0:c1], Act.Sigmoid)
                    sn = sbuf_pool.tile([P, n], f32, name=f"zn{rep}_{w}_{c0}",
                                        tag="zch", bufs=4)
                    nc.scalar.activation(sn[:], e_win[:, c0:c1], Act.Sigmoid,
                                         scale=-1.0)
                    rn = sbuf_pool.tile([P, n], f32, name=f"zr{rep}_{w}_{c0}",
                                        tag="zch", bufs=4)
                    nc.vector.reciprocal(rn[:], sn[:])
                    nc.vector.tensor_tensor(z_win[:, c0:c1], sp[:], rn[:],
                                            Alu.mult)
                    # two-term z: z8 = fp8-rounded z (kept in f32 so the scalar
                    # multiply with exact one-hots stores exactly in fp8),
                    # zlo = z - z8 (fp8-stored residual)
                    z8_win, zlo_win = st[w]["z8_win"], st[w]["zlo_win"]
                    z8q = sbuf_pool.tile([P, n], fp8, name=f"zq{rep}_{w}_{c0}",
                                         tag="z8q", bufs=4)
                    nc.vector.tensor_copy(z8q[:], z_win[:, c0:c1])
                    nc.vector.tensor_copy(z8_win[:, c0:c1], z8q[:])
                    nc.vector.tensor_tensor(zlo_win[:, c0:c1], z_win[:, c0:c1],
                                            z8_win[:, c0:c1], Alu.subtract)

                def emit_wsum_pair(w, j):
                    wsum_ps = st[w]["wsum_ps"]
                    z8_win, zlo_win = st[w]["z8_win"], st[w]["zlo_win"]
                    if j == 0:
                        # wsum_lo reuses the fv bank (same pool tag, bufs=1)
                        st[w]["wlo_ps"] = fv_ps_pool.tile(
                            [P, D], f32, name=f"wlo{rep}_{w}", tag="fv_ps")
                    wlo_ps = st[w]["wlo_ps"]
                    ohz = sbuf_pool.tile([P, 2, P], fp8, name=f"oz{rep}_{w}_{j}",
                                         tag="ohz", bufs=4)
                    ohzlo = sbuf_pool.tile([P, 2, P], fp8, name=f"ol{rep}_{w}_{j}",
                                           tag="ohzlo", bufs=4)
                    for k in range(2):
                        t = 2 * j + k
                        nc.vector.tensor_scalar(ohz[:, k, :], ohw_t(w, t),
                                                z8_win[:, t:t + 1], None, Alu.mult)
                        nc.vector.tensor_scalar(ohzlo[:, k, :], ohw_t(w, t),
                                                zlo_win[:, t:t + 1], None,
                                                Alu.mult)
                    nc.tensor.matmul(wsum_ps[:, 0:D], ohz[:], nat_pair(w, j),
                                     start=(j == 0), stop=(j == NP - 1),
                                     perf_mode=PM.DoubleRow)
                    # NOTE: further accumulation groups in the same PSUM bank must
                    # ride the first group's start/stop envelope (start=False;
                    # the j==0 start above zeroes the whole bank) — interleaved
                    # groups with their own start wipe the bank's other region.
                    nc.tensor.matmul(wsum_ps[:, D:D + 1], ohz[:],
                                     one_sb[:].unsqueeze(2),
                                     start=False, stop=False,
                                     perf_mode=PM.DoubleRow,
                                     skip_group_check=True)
                    nc.tensor.matmul(wsum_ps[:, D:D + 1], ohzlo[:],
                                     one_sb[:].unsqueeze(2),
                                     start=False, stop=(j == NP - 1),
                                     perf_mode=PM.DoubleRow,
                                     skip_group_check=True)
                    nc.tensor.matmul(wlo_ps[:], ohzlo[:], nat_pair(w, j),
                                     start=(j == 0), stop=(j == NP - 1),
                                     perf_mode=PM.DoubleRow)

                def emit_output(w):
                    wsum_ps, out_sb = st[w]["wsum_ps"], st[w]["out_sb"]
                    wlo_ps = st[w]["wlo_ps"]
                    den = sbuf_pool.tile([P, 1], f32, name=f"dn{rep}_{w}",
                                         tag="den", bufs=2)
                    nc.vector.tensor_scalar(den[:], wsum_ps[:, D:D + 1], 1e-30,
                                            None, Alu.max)
                    rden = sbuf_pool.tile([P, 1], f32, name=f"rd{rep}_{w}",
                                          tag="rden", bufs=2)
                    nc.vector.reciprocal(rden[:], den[:])
                    wlo_sc = sbuf_pool.tile([P, D], f32, name=f"wl{rep}_{w}",
                                            tag="wlo_sc", bufs=2)
                    nc.scalar.mul(wlo_sc[:], wlo_ps[:], rden[:])
                    nc.vector.scalar_tensor_tensor(
                        out=out_sb[:, 0:D], in0=wsum_ps[:, 0:D], scalar=rden[:],
                        in1=wlo_sc[:], op0=Alu.mult, op1=Alu.add)
                    nc.sync.dma_start(out_dram[w], out_sb[:])
                    if _DEBUG:
                        dbg = sbuf_pool.tile([P, 2 * T_W], f32,
                                             name=f"dbg{rep}_{w}", tag="dbg",
                                             bufs=2)
                        nc.vector.tensor_copy(dbg[:, 0:T_W], st[w]["e_win"][:])
                        nc.vector.tensor_copy(dbg[:, T_W:], st[w]["z_win"][:])
                        nc.sync.dma_start(dbg_dram[w], dbg[:])

                # ---------------- emission schedule ----------------
                # Software-pipelined: e-matmuls lag their sigmoid batch by
                # E_LAG so the in-order PE stream never stalls on Act; z runs
                # in fine-grained chunks (ZB batches) so wsum pairs drain
                # incrementally; the small leftover tail of window 0 overlaps
                # window 1's anchor pass (anchor banks alternate by parity).
                E_LAG = 2
                ZB = 3
                bd = list(range(0, NB, ZB)) + [NB]
                if bd[-2] == NB:
                    bd.pop()

                def pass2b(w):
                    from collections import deque
                    seq, ready = [], deque()

                    def drain(n):
                        for _ in range(n):
                            if ready:
                                seq.append(ready.popleft())

                    def after_e(eb):
                        seq.append(("e", w, eb))
                        drain(2)
                        if (eb + 1) in bd[1:]:
                            c = bd.index(eb + 1) - 1
                            seq.append(("z", w, c))
                            for j in range(bd[c] * 2, bd[c + 1] * 2):
                                ready.append(("w", w, j))

                    for b in range(NB):
                        seq.append(("s", w, b))
                        drain(2)
                        if b - E_LAG >= 0:
                            after_e(b - E_LAG)
                    for eb in range(max(0, NB - E_LAG), NB):
                        after_e(eb)
                    return seq, list(ready)

                def dispatch(seq):
                    for kind, w, i in seq:
                        if kind == "s":
                            emit_s_matmuls(w, i)
                        elif kind == "e":
                            emit_e_batch(w, i)
                        elif kind == "z":
                            emit_z_chunk(w, bd[i] * SB, bd[i + 1] * SB)
                        elif kind == "w":
                            emit_wsum_pair(w, i)
                        elif kind == "a":
                            emit_anchor_pair(w, i)

                emit_window_setup(0)
                emit_window_setup(1)
                for j in range(NP):
                    emit_anchor_pair(0, j)
                emit_fv(0)
                seq0, tail0 = pass2b(0)
                dispatch(seq0)
                dispatch(_weave2([("a", 1, j) for j in range(NP)], tail0))
                emit_output(0)
                emit_fv(1)
                seq1, tail1 = pass2b(1)
                dispatch(seq1)
                dispatch(tail1)
                emit_output(1)

            if loop_repeat is not None:
                with tc.For_i(0, loop_repeat, 1):
                    body("L")
            else:
                for rep in range(repeat):
                    body(rep)

    return nc


def _weave2(a_items, b_items):
    """Proportionally interleave two lists, preserving each list's order."""
    na, nb = len(a_items), len(b_items)
    out = []
    ai = bi = 0
    while ai < na or bi < nb:
        if ai < na and (bi >= nb or ai * nb <= bi * na):
            out.append(a_items[ai]); ai += 1
        else:
            out.append(b_items[bi]); bi += 1
    return out


def _prepare(ifeat, Wu, Wv, bv, we, seg_ids):
    """Host-side shard + pad + layout. Returns (T_W, in_maps)."""
    ifeat = np.asarray(ifeat, dtype=np.float32)
    Wu = np.asarray(Wu, dtype=np.float32)
    Wv = np.asarray(Wv, dtype=np.float32)
    bv = np.asarray(bv, dtype=np.float32)
    we = np.asarray(we, dtype=np.float32)
    seg_ids = np.asarray(seg_ids)

    W = N_WINDOWS
    bounds = np.searchsorted(
        seg_ids, np.arange(0, B + 1, SEGS_PER_WINDOW), side="left")
    n_w = np.diff(bounds)
    T_W = max(4, int(-(-int(n_w.max()) // P)))
    T_W = ((T_W + 3) // 4) * 4
    NT = W_PER_CORE * T_W

    win = (seg_ids // SEGS_PER_WINDOW).astype(np.int64)
    pos = np.arange(N, dtype=np.int64) - bounds[win]
    sloc = (seg_ids % SEGS_PER_WINDOW).astype(np.int64)

    if8 = ifeat.astype(F8)
    # error-diffused fp8 rounding of nat: within each segment (nodes sorted),
    # carry the rounding residual forward so segment sums are nearly exact.
    natq = np.empty((N, D), dtype=F8)
    carry = np.zeros(D, dtype=np.float32)
    seg_np = np.asarray(seg_ids, dtype=np.int64)
    prev = -1
    for i in range(N):
        s = seg_np[i]
        if s != prev:
            carry[:] = 0.0
            prev = s
        v = ifeat[i] + carry
        q = v.astype(F8)
        carry = v - q.astype(np.float32)
        natq[i] = q
    natA = np.zeros((W, T_W * P, D), dtype=F8)
    natA[win, pos, :] = natq
    ifA = np.zeros((W, T_W * P, D), dtype=F8)   # nearest-rounded for fc_u
    ifA[win, pos, :] = if8
    ohwA = np.zeros((W, T_W * P, P), dtype=F8)
    ohwA[win, pos, sloc] = 1.0

    counts = np.bincount(np.asarray(seg_ids, dtype=np.int64), minlength=B)
    rcnt = (1.0 / np.maximum(counts, 1)).astype(np.float32).reshape(W, P, 1)

    wuT8 = (np.ascontiguousarray(Wu.T) * WU_SCALE).reshape(2, P, D)
    wuT8 = np.ascontiguousarray(wuT8.transpose(1, 0, 2)).astype(F8)  # [P,2,D]
    wvT8 = (np.ascontiguousarray(Wv.T) * WU_SCALE).reshape(2, P, D)
    wvT8 = np.ascontiguousarray(wvT8.transpose(1, 0, 2)).astype(F8)
    web = np.ascontiguousarray(we.reshape(2, P).T).astype(BF)  # [dlo, half]
    bvb8 = np.tile(bv * FV_SCALE, (P, 1)).astype(np.float32)
    idb = np.eye(P, dtype=BF)
    one8 = np.ones((P, 2), dtype=F8)

    in_maps = []
    for c in range(N_CORES):
        X = natA[2 * c:2 * c + 2].reshape(W_PER_CORE, T_W, P, D)
        # nat [lane, (w,t), d]
        natp = np.ascontiguousarray(
            X.transpose(2, 0, 1, 3).reshape(P, NT, D))
        # ifT [d_lo, (w,t), kb, lane]
        Y = ifA[2 * c:2 * c + 2].reshape(W_PER_CORE, T_W, P, 2, P)
        iftp = np.ascontiguousarray(
            Y.transpose(4, 0, 1, 3, 2).reshape(P, NT, 2, P))
        O = ohwA[2 * c:2 * c + 2].reshape(W_PER_CORE, T_W, P, P)
        ohwp = np.ascontiguousarray(
            O.transpose(2, 0, 1, 3).reshape(P, NT, P))
        ohtp = np.ascontiguousarray(
            O.transpose(3, 0, 1, 2).reshape(P, NT, P))
        in_maps.append({
            "natp": natp, "iftp": iftp, "ohwp": ohwp, "ohtp": ohtp,
            "wuT8": wuT8, "wvT8": wvT8, "web": web, "bvb8": bvb8,
            "idb": idb, "one8": one8,
            "rcnt": rcnt[2 * c:2 * c + 2],
        })
    return T_W, in_maps


_DEBUG = False
_LAST = {}


def _run(ifeat, Wu, Wv, bv, we, seg_ids, trace=False):
    from concourse.bass_utils import run_bass_kernel_spmd

    T_W, in_maps = _prepare(ifeat, Wu, Wv, bv, we, seg_ids)
    nc = _build(T_W)
    _split_sync_waits(nc)
    res = run_bass_kernel_spmd(nc, in_maps, list(range(N_CORES)), trace=trace)
    _LAST["res"] = res
    _LAST["T_W"] = T_W
    _LAST["nc"] = nc
    _LAST["in_maps"] = in_maps

    out = np.empty((B, 2 * D), dtype=np.float32)
    for c in range(N_CORES):
        core_out = res.results[c]["out"]  # [W_PER_CORE, P, 2D]
        for wl in range(W_PER_CORE):
            w = c * W_PER_CORE + wl
            out[w * SEGS_PER_WINDOW:(w + 1) * SEGS_PER_WINDOW, :] = core_out[wl]
    return out


def kernel(ifeat, Wu, Wv, bv, we, seg_ids):
    return _run(ifeat, Wu, Wv, bv, we, seg_ids, trace=False)


# revision 40
# speedup vs baseline: 2.1783x; 2.1783x over previous
"""Trainium2 Bass kernel for nn_AttnReadout (segment attention readout).

Computation (reference):
    anchor[b]  = mean of ifeat rows in segment b                  [B, D]
    e[i]       = sigmoid(ifeat @ Wu.T + (anchor @ Wv.T + bv)[seg]) @ we
    alpha      = segment_softmax(e)
    rst[b]     = sum_i alpha[i] * ifeat[i]                        [B, D]
    out        = concat([rst, anchor], axis=1)                    [B, 2D]

Sharding: 2048 segments -> 8 cores x 2 windows of 128 contiguous segments.
Nodes (sorted by segment) are padded per-window to T_W tiles of 128 rows.

All heavy matmuls run in fp8e4 with MatmulPerfMode.DoubleRow (2 k-tiles per
instruction, 0.5 cycles per output column):
  - anchor / weighted-sum segment reductions pair ADJACENT node tiles as the
    two k-tiles (one-hot pair as stationary, feature pair as moving).
  - fc_u contracts the two 128-feature halves as k-tiles.
  - the fv gather duplicates its single 128-seg k-tile via stride-0 APs and
    pre-halves fv to compensate.
One-hot matrices in both layouts (ohw [node,seg] and ohT [seg,node]) are
host-prepared from seg_ids and DMAed as fp8 (no on-device transposes).
Sigmoid runs on batches of 4 node tiles from a 2-bank PSUM region to
amortize the activation-engine access latency; z = exp(e) uses the
sigmoid(e)/sigmoid(-e) identity to stay on one ACT table.

Scaling: Wu.T and Wv.T are stored x16 (good fp8 range), anchor means x4,
fv stored as 8*(fv+bv) (halved for the stride-0 duplication), and the
sigmoid applies scale=1/16 to undo it. The z scale cancels in rst/denom.
"""

import numpy as np
import ml_dtypes

N = 102400
D = 256
B = 2048
N_CORES = 8
W_PER_CORE = 2
N_WINDOWS = N_CORES * W_PER_CORE  # 16
SEGS_PER_WINDOW = B // N_WINDOWS  # 128
P = 128
BF = ml_dtypes.bfloat16
F8 = ml_dtypes.float8_e4m3fn

WU_SCALE = 16.0
ANCH_SCALE = 4.0
FV_SCALE = 8.0  # fv stored as FV_SCALE*(fv+bv); doubled by dup -> 16
SB = 4          # sigmoid batch (node tiles per activation)


def _apply_tile_patch():
    """Split TileContext's multi-wait tail drain into single-wait drains
    (this walrus build rejects >1 sync wait on a Drain instruction)."""
    import concourse.tile as tile_mod
    from concourse.vector_clock import ScopedClock

    if getattr(tile_mod.TileContext, "_drain_wait_split_patch", False):
        return

    def _patched(self, tick_clock, wait_clock):
        nc = self.nc
        drain_inst = nc.sync.drain()
        wait_clock.add_sem_waits(
            drain_inst.ins, ScopedClock({None: tick_clock.global_clock})
        )
        si = drain_inst.ins.sync_info
        waits = list(si.on_wait) if si is not None else []
        if len(waits) > 1:
            SyncInfo = type(si)
            drain_inst.ins.sync_info = SyncInfo(
                on_wait=[waits[0]], on_update=list(si.on_update)
            )
            for w in waits[1:]:
                extra = nc.sync.drain()
                extra.ins.sync_info = SyncInfo(on_wait=[w], on_update=[])

        nc.all_engine_barrier()
        assert self.sems is not None
        popped = nc._tile_sem_poison_stack.pop()
        assert popped is self._sem_poison
        nc.clear_and_free_semaphores(list(self.sems.allocated().values()))
        nc.all_engine_barrier()

    tile_mod.TileContext._drain_and_barrier = _patched
    tile_mod.TileContext._drain_wait_split_patch = True


def _split_sync_waits(nc, limit=1):
    """Split >limit sync waits per instruction into preceding single-wait
    EventSemaphore carriers on the same engine (walrus build limit)."""
    import concourse.mybir as mybir

    n_new = 0
    for _, bassbb in nc.bb_map.items():
        insts = bassbb.bb.instructions  # live list
        snapshot = list(insts)
        offset = 0
        for pos, inst in enumerate(snapshot):
            si = getattr(inst, "sync_info", None)
            if si is None:
                continue
            waits = list(si.on_wait)
            if len(waits) <= limit:
                continue
            SyncInfo = type(si)
            inst.sync_info = SyncInfo(
                on_wait=waits[:limit], on_update=list(si.on_update))
            carriers = []
            for w in waits[limit:]:
                c = mybir.InstEventSemaphore(
                    name=f"WSPLIT-{nc.next_id()}", ins=[], outs=[])
                c.engine = inst.engine
                c.sync_info = SyncInfo(on_wait=[w], on_update=[])
                carriers.append(c)
            insts[pos + offset:pos + offset] = carriers
            offset += len(carriers)
            n_new += len(carriers)
    return n_new


_PACK = {}
_STAGGER = False
_UNROLL = 1


def _build(T_W, repeat=1, loop_repeat=None):
    """Build the single-core SPMD Bass program; T_W must be a multiple of 4.

    Uses _PACK["bases"] (canonical per-tile 32-aligned segment-block base,
    identical across cores) and _PACK["width"] set by _prepare: the one-hot
    of tile t only has nonzero columns in [base(t), base(t)+width)."""
    import contextlib
    import concourse.bass as bass
    import concourse.mybir as mybir
    from concourse.tile import TileContext

    _apply_tile_patch()

    f32 = mybir.dt.float32
    bf16 = mybir.dt.bfloat16
    fp8 = mybir.dt.float8e4
    Alu = mybir.AluOpType
    Act = mybir.ActivationFunctionType
    PM = mybir.MatmulPerfMode

    assert T_W % 4 == 0
    CH = T_W // 2            # tiles per DMA chunk (2 chunks per window), even
    NT = W_PER_CORE * T_W
    NB = T_W // SB           # sigmoid batches per window
    NB0 = NB // 2            # batches in z-chunk 0
    TC0 = NB0 * SB           # tiles in z-chunk 0
    NP = T_W // 2            # node-tile pairs per window
    NP0 = TC0 // 2           # pairs fully covered by z-chunk 0

    nc = bass.Bass("TRN2", num_devices=N_CORES)

    nat_dram = nc.dram_tensor("natp", [P, NT, D], fp8, kind="ExternalInput")
    ifT_dram = nc.dram_tensor("iftp", [P, NT, 2, P], fp8, kind="ExternalInput")
    ohw_dram = nc.dram_tensor("ohwp", [P, NT, P], fp8, kind="ExternalInput")
    ohT_dram = nc.dram_tensor("ohtp", [P, NT, P], fp8, kind="ExternalInput")
    wuT_dram = nc.dram_tensor("wuT8", [P, 2, D], fp8, kind="ExternalInput")
    wvT_dram = nc.dram_tensor("wvT8", [P, 2, D], fp8, kind="ExternalInput")
    web_dram = nc.dram_tensor("web", [P, 2], bf16, kind="ExternalInput")
    bvb_dram = nc.dram_tensor("bvb8", [P, D], f32, kind="ExternalInput")
    idb_dram = nc.dram_tensor("idb", [P, P], bf16, kind="ExternalInput")
    one_dram = nc.dram_tensor("one8", [P, 2], fp8, kind="ExternalInput")
    rcnt_dram = nc.dram_tensor("rcnt", [W_PER_CORE, P, 1], f32,
                               kind="ExternalInput")
    out_dram = nc.dram_tensor("out", [W_PER_CORE, P, 2 * D], f32,
                              kind="ExternalOutput")
    dbg_dram = None
    if _DEBUG:
        dbg_dram = nc.dram_tensor("dbg", [W_PER_CORE, P, 2 * T_W], f32,
                                  kind="ExternalOutput")

    with TileContext(nc) as tc:
        with contextlib.ExitStack() as ctx:
            const_pool = ctx.enter_context(tc.tile_pool(name="const", bufs=1))
            data_pool = ctx.enter_context(tc.tile_pool(name="data", bufs=1))
            sbuf_pool = ctx.enter_context(tc.tile_pool(name="sbuf", bufs=1))
            # PSUM: s_ps 2x2 banks + anchor 1 (tile also hosts the e columns
            # and transpose scratch) + wsum 1 + fv/wlo 2 (cross-rep overlap:
            # next rep's fv must not wait on this rep's final output) = 8.
            anchor_ps_pool = ctx.enter_context(
                tc.tile_pool(name="anchor_ps", bufs=1, space="PSUM"))
            wsum_ps_pool = ctx.enter_context(
                tc.tile_pool(name="wsum_ps", bufs=1, space="PSUM"))
            s_ps_pool = ctx.enter_context(
                tc.tile_pool(name="s_ps", bufs=2, space="PSUM"))
            fv_ps_pool = ctx.enter_context(
                tc.tile_pool(name="fv_ps", bufs=2, space="PSUM"))

            # ---- constants ----
            wuT_sb = const_pool.tile([P, 2, D], fp8, name="wuT_sb", tag="wuT_sb")
            nc.sync.dma_start(wuT_sb[:], wuT_dram[:])
            wvT_sb = const_pool.tile([P, 2, D], fp8, name="wvT_sb", tag="wvT_sb")
            nc.sync.dma_start(wvT_sb[:], wvT_dram[:])
            web_sb = const_pool.tile([P, 2], bf16, name="web_sb", tag="web_sb")
            nc.sync.dma_start(web_sb[:], web_dram[:])
            bvb_sb = const_pool.tile([P, D], f32, name="bvb_sb", tag="bvb_sb")
            nc.sync.dma_start(bvb_sb[:], bvb_dram[:])
            idb_sb = const_pool.tile([P, P], bf16, name="idb_sb", tag="idb_sb")
            nc.sync.dma_start(idb_sb[:], idb_dram[:])
            one_sb = const_pool.tile([P, 2], fp8, name="one_sb", tag="one_sb")
            nc.sync.dma_start(one_sb[:], one_dram[:])
            rcnt_sb = const_pool.tile([P, W_PER_CORE, 1], f32, name="rcnt_sb",
                                      tag="rcnt_sb")
            nc.sync.dma_start(
                rcnt_sb[:], rcnt_dram[:].rearrange("w p one -> p w one"))

            def body(rep):
                # window data, 2 chunks per window, all resident.
                # DMA emission follows consumption order: the anchor pass of a
                # window needs ohw+nat of both its chunks first; ifT/ohT feed
                # the later s-pipeline.
                nat_ch, ifT_ch, ohw_ch, ohT_ch = {}, {}, {}, {}

                def dma_ohw_nat(c):
                    ohwc = data_pool.tile([P, CH, P], fp8,
                                          name=f"ohwc{rep}_{c}", tag="ohwc",
                                          bufs=4)
                    nc.sync.dma_start(ohwc[:], ohw_dram[:, c * CH:(c + 1) * CH, :])
                    ohw_ch[c] = ohwc
                    natc = data_pool.tile([P, CH, D], fp8,
                                          name=f"natc{rep}_{c}", tag="natc",
                                          bufs=4)
                    nc.sync.dma_start(natc[:], nat_dram[:, c * CH:(c + 1) * CH, :])
                    nat_ch[c] = natc

                def dma_ift_oht(c):
                    iftc = data_pool.tile([P, CH, 2, P], fp8,
                                          name=f"iftc{rep}_{c}", tag="iftc",
                                          bufs=4)
                    nc.sync.dma_start(iftc[:], ifT_dram[:, c * CH:(c + 1) * CH, :, :])
                    ifT_ch[c] = iftc
                    ohtc = data_pool.tile([P, CH, P], fp8,
                                          name=f"ohtc{rep}_{c}", tag="ohtc",
                                          bufs=4)
                    nc.sync.dma_start(ohtc[:], ohT_dram[:, c * CH:(c + 1) * CH, :])
                    ohT_ch[c] = ohtc

                dma_ohw_nat(0)
                dma_ohw_nat(1)
                dma_ift_oht(0)
                dma_ift_oht(1)
                dma_ohw_nat(2)
                dma_ohw_nat(3)
                dma_ift_oht(2)
                dma_ift_oht(3)

                def nat_pair(w, j):      # [P, 2, D] node-tile pair
                    g = w * T_W + 2 * j
                    return nat_ch[g // CH][:, (g % CH):(g % CH) + 2, :]

                def ohw_pair(w, j):      # [P, 2, P]
                    g = w * T_W + 2 * j
                    return ohw_ch[g // CH][:, (g % CH):(g % CH) + 2, :]

                def ifT_t(w, t):         # [P, 2, P]
                    g = w * T_W + t
                    return ifT_ch[g // CH][:, g % CH, :, :]

                def ohT_t(w, t):         # [P, P]
                    g = w * T_W + t
                    return ohT_ch[g // CH][:, g % CH, :]

                def ohw_t(w, t):         # [P, P]
                    g = w * T_W + t
                    return ohw_ch[g // CH][:, g % CH, :]

                # per-window state
                st = {}

                def emit_anchor_pair(w, j):
                    nc.tensor.matmul(st[w]["anchor_ps"][:, 0:D], ohw_pair(w, j),
                                     nat_pair(w, j), start=(j == 0),
                                     stop=(j == NP - 1), perf_mode=PM.DoubleRow)

                def emit_window_setup(w):
                    # anchor bank also hosts the per-batch e columns [D:D+T_W]
                    # and the two bf16 transpose scratch regions after them
                    anchor_ps = anchor_ps_pool.tile(
                        [P, D + T_W + P], f32, name=f"anc{rep}_{w}",
                        tag="anchor_ps")
                    out_sb = sbuf_pool.tile([P, 2 * D], f32, name=f"osb{rep}_{w}",
                                            tag="out_sb", bufs=2)
                    e_win = sbuf_pool.tile([P, T_W], f32, name=f"ew{rep}_{w}",
                                           tag="e_win", bufs=2)
                    z_win = sbuf_pool.tile([P, T_W], f32, name=f"zw{rep}_{w}",
                                           tag="z_win", bufs=2)
                    z8_win = sbuf_pool.tile([P, T_W], f32, name=f"z8{rep}_{w}",
                                            tag="z8_win", bufs=2)
                    zlo_win = sbuf_pool.tile([P, T_W], f32, name=f"zl{rep}_{w}",
                                             tag="zlo_win", bufs=2)
                    wsum_ps = wsum_ps_pool.tile([P, D + 1], f32,
                                                name=f"ws{rep}_{w}", tag="wsum_ps")
                    st[w] = dict(anchor_ps=anchor_ps, out_sb=out_sb,
                                 e_win=e_win, z_win=z_win, z8_win=z8_win,
                                 zlo_win=zlo_win, wsum_ps=wsum_ps)

                def emit_fv(w):
                    anchor_ps = st[w]["anchor_ps"]
                    rc = rcnt_sb[:, w, :]
                    # anchor output half (Act engine keeps DVE free; reads PSUM)
                    nc.scalar.mul(st[w]["out_sb"][:, D:2 * D], anchor_ps[:, 0:D],
                                  rc)
                    anchb = sbuf_pool.tile([P, D], bf16, name=f"a8{rep}_{w}",
                                           tag="anchb", bufs=1)
                    nc.vector.tensor_scalar(anchb[:], anchor_ps[:, 0:D], rc,
                                            ANCH_SCALE, Alu.mult, Alu.mult)
                    anchT = sbuf_pool.tile([P, 2, P], fp8, name=f"aT{rep}_{w}",
                                           tag="anchT", bufs=1)
                    trbase = D + T_W
                    for db in range(2):
                        trp = (anchor_ps[:, trbase + db * (P // 2):
                                         trbase + (db + 1) * (P // 2)]
                               .bitcast(bf16))
                        nc.tensor.transpose(trp, anchb[:, db * P:(db + 1) * P],
                                            idb_sb[:])
                        nc.scalar.copy(anchT[:, db, :], trp)
                    fv_ps = fv_ps_pool.tile([P, D], f32, name=f"fvp{rep}_{w}",
                                            tag="fv_ps")
                    nc.tensor.matmul(fv_ps[:], anchT[:], wvT_sb[:],
                                     start=True, stop=True, perf_mode=PM.DoubleRow)
                    fv8 = sbuf_pool.tile([P, D], fp8, name=f"fv8{rep}_{w}",
                                         tag="fv8", bufs=1)
                    # fv8 = fv_ps * (FV_SCALE / (WU_SCALE*ANCH_SCALE)) + bv*FV_SCALE
                    nc.vector.scalar_tensor_tensor(
                        out=fv8[:], in0=fv_ps[:],
                        scalar=FV_SCALE / (WU_SCALE * ANCH_SCALE), in1=bvb_sb[:],
                        op0=Alu.mult, op1=Alu.add)
                    st[w]["fv8"] = fv8

                def emit_s_matmuls(w, b):
                    # transposed s: sT[do_half, node] per (half, q); the e
                    # reduction is then a ~free stationary-sT matmul with the
                    # we column as the 1-wide moving operand.
                    s_ps = s_ps_pool.tile([P, 2, SB, P], f32,
                                          name=f"sp{rep}_{w}_{b}", tag="s_ps")
                    fv8 = st[w]["fv8"]
                    for q in range(SB):
                        t = b * SB + q
                        oht_dup = ohT_t(w, t).unsqueeze(1).broadcast_to([P, 2, P])
                        for h in range(2):
                            nc.tensor.matmul(s_ps[:, h, q, :],
                                             wuT_sb[:, :, h * P:(h + 1) * P],
                                             ifT_t(w, t), start=True, stop=False,
                                             perf_mode=PM.DoubleRow)
                            fv_dup = (fv8[:, h * P:(h + 1) * P]
                                      .unsqueeze(1).broadcast_to([P, 2, P]))
                            nc.tensor.matmul(s_ps[:, h, q, :], fv_dup, oht_dup,
                                             start=False, stop=True,
                                             perf_mode=PM.DoubleRow)
                    s_sb = sbuf_pool.tile([P, 2, SB, P], bf16,
                                          name=f"ss{rep}_{w}_{b}", tag="s_sb",
                                          bufs=3)
                    nc.scalar.activation(
                        s_sb[:].rearrange("p h q d -> p (h q d)"),
                        s_ps[:].rearrange("p h q d -> p (h q d)"),
                        Act.Sigmoid, scale=1.0 / WU_SCALE)
                    st[w][f"s_sb{b}"] = s_sb

                def emit_e_batch(w, b):
                    anchor_ps = st[w]["anchor_ps"]
                    s_sb = st[w].pop(f"s_sb{b}")
                    for q in range(SB):
                        ec = D + b * SB + q
                        for h in range(2):
                            nc.tensor.matmul(anchor_ps[:, ec:ec + 1],
                                             s_sb[:, h, q, :],
                                             web_sb[:, h:h + 1],
                                             start=(h == 0), stop=(h == 1))
                    nc.vector.tensor_copy(
                        st[w]["e_win"][:, b * SB:(b + 1) * SB],
                        anchor_ps[:, D + b * SB:D + (b + 1) * SB])

                def emit_z_chunk(w, c0, c1):
                    e_win, z_win = st[w]["e_win"], st[w]["z_win"]
                    n = c1 - c0
                    sp = sbuf_pool.tile([P, n], f32, name=f"zp{rep}_{w}_{c0}",
                                        tag="zch", bufs=4)
                    nc.scalar.activation(sp[:], e_win[:, c0:c1], Act.Sigmoid)
                    sn = sbuf_pool.tile([P, n], f32, name=f"zn{rep}_{w}_{c0}",
                                        tag="zch", bufs=4)
                    nc.scalar.activation(sn[:], e_win[:, c0:c1], Act.Sigmoid,
                                         scale=-1.0)
                    rn = sbuf_pool.tile([P, n], f32, name=f"zr{rep}_{w}_{c0}",
                                        tag="zch", bufs=4)
                    nc.vector.reciprocal(rn[:], sn[:])
                    nc.vector.tensor_tensor(z_win[:, c0:c1], sp[:], rn[:],
                                            Alu.mult)
                    # two-term z: z8 = fp8-rounded z (kept in f32 so the scalar
                    # multiply with exact one-hots stores exactly in fp8),
                    # zlo = z - z8 (fp8-stored residual)
                    z8_win, zlo_win = st[w]["z8_win"], st[w]["zlo_win"]
                    z8q = sbuf_pool.tile([P, n], fp8, name=f"zq{rep}_{w}_{c0}",
                                         tag="z8q", bufs=4)
                    nc.vector.tensor_copy(z8q[:], z_win[:, c0:c1])
                    nc.vector.tensor_copy(z8_win[:, c0:c1], z8q[:])
                    nc.vector.tensor_tensor(zlo_win[:, c0:c1], z_win[:, c0:c1],
                                            z8_win[:, c0:c1], Alu.subtract)

                def emit_wsum_pair(w, j):
                    wsum_ps = st[w]["wsum_ps"]
                    z8_win, zlo_win = st[w]["z8_win"], st[w]["zlo_win"]
                    if j == 0:
                        # wsum_lo reuses the fv bank (same pool tag, bufs=1)
                        st[w]["wlo_ps"] = fv_ps_pool.tile(
                            [P, D], f32, name=f"wlo{rep}_{w}", tag="fv_ps")
                    wlo_ps = st[w]["wlo_ps"]
                    ohz = sbuf_pool.tile([P, 2, P], fp8, name=f"oz{rep}_{w}_{j}",
                                         tag="ohz", bufs=4)
                    ohzlo = sbuf_pool.tile([P, 2, P], fp8, name=f"ol{rep}_{w}_{j}",
                                           tag="ohzlo", bufs=4)
                    for k in range(2):
                        t = 2 * j + k
                        nc.vector.tensor_scalar(ohz[:, k, :], ohw_t(w, t),
                                                z8_win[:, t:t + 1], None, Alu.mult)
                        nc.vector.tensor_scalar(ohzlo[:, k, :], ohw_t(w, t),
                                                zlo_win[:, t:t + 1], None,
                                                Alu.mult)
                    nc.tensor.matmul(wsum_ps[:, 0:D], ohz[:], nat_pair(w, j),
                                     start=(j == 0), stop=(j == NP - 1),
                                     perf_mode=PM.DoubleRow)
                    # NOTE: further accumulation groups in the same PSUM bank must
                    # ride the first group's start/stop envelope (start=False;
                    # the j==0 start above zeroes the whole bank) — interleaved
                    # groups with their own start wipe the bank's other region.
                    nc.tensor.matmul(wsum_ps[:, D:D + 1], ohz[:],
                                     one_sb[:].unsqueeze(2),
                                     start=False, stop=False,
                                     perf_mode=PM.DoubleRow,
                                     skip_group_check=True)
                    nc.tensor.matmul(wsum_ps[:, D:D + 1], ohzlo[:],
                                     one_sb[:].unsqueeze(2),
                                     start=False, stop=(j == NP - 1),
                                     perf_mode=PM.DoubleRow,
                                     skip_group_check=True)
                    nc.tensor.matmul(wlo_ps[:], ohzlo[:], nat_pair(w, j),
                                     start=(j == 0), stop=(j == NP - 1),
                                     perf_mode=PM.DoubleRow)

                def emit_output(w):
                    wsum_ps, out_sb = st[w]["wsum_ps"], st[w]["out_sb"]
                    wlo_ps = st[w]["wlo_ps"]
                    den = sbuf_pool.tile([P, 1], f32, name=f"dn{rep}_{w}",
                                         tag="den", bufs=2)
                    nc.vector.tensor_scalar(den[:], wsum_ps[:, D:D + 1], 1e-30,
                                            None, Alu.max)
                    rden = sbuf_pool.tile([P, 1], f32, name=f"rd{rep}_{w}",
                                          tag="rden", bufs=2)
                    nc.vector.reciprocal(rden[:], den[:])
                    wlo_sc = sbuf_pool.tile([P, D], f32, name=f"wl{rep}_{w}",
                                            tag="wlo_sc", bufs=2)
                    nc.scalar.mul(wlo_sc[:], wlo_ps[:], rden[:])
                    nc.vector.scalar_tensor_tensor(
                        out=out_sb[:, 0:D], in0=wsum_ps[:, 0:D], scalar=rden[:],
                        in1=wlo_sc[:], op0=Alu.mult, op1=Alu.add)
                    nc.sync.dma_start(out_dram[w], out_sb[:])
                    if _DEBUG:
                        dbg = sbuf_pool.tile([P, 2 * T_W], f32,
                                             name=f"dbg{rep}_{w}", tag="dbg",
                                             bufs=2)
                        nc.vector.tensor_copy(dbg[:, 0:T_W], st[w]["e_win"][:])
                        nc.vector.tensor_copy(dbg[:, T_W:], st[w]["z_win"][:])
                        nc.sync.dma_start(dbg_dram[w], dbg[:])

                # ---------------- emission schedule ----------------
                # Software-pipelined: e-matmuls lag their sigmoid batch by
                # E_LAG so the in-order PE stream never stalls on Act; z runs
                # in fine-grained chunks (ZB batches) so wsum pairs drain
                # incrementally; the small leftover tail of window 0 overlaps
                # window 1's anchor pass (anchor banks alternate by parity).
                E_LAG = 2
                ZB = 3
                bd = list(range(0, NB, ZB)) + [NB]
                if bd[-2] == NB:
                    bd.pop()

                def pass2b(w):
                    from collections import deque
                    seq, ready = [], deque()

                    def drain(n):
                        for _ in range(n):
                            if ready:
                                seq.append(ready.popleft())

                    def after_e(eb):
                        seq.append(("e", w, eb))
                        drain(2)
                        if (eb + 1) in bd[1:]:
                            c = bd.index(eb + 1) - 1
                            seq.append(("z", w, c))
                            for j in range(bd[c] * 2, bd[c + 1] * 2):
                                ready.append(("w", w, j))

                    for b in range(NB):
                        seq.append(("s", w, b))
                        drain(2)
                        if b - E_LAG >= 0:
                            after_e(b - E_LAG)
                    for eb in range(max(0, NB - E_LAG), NB):
                        after_e(eb)
                    return seq, list(ready)

                def dispatch(seq):
                    for kind, w, i in seq:
                        if kind == "s":
                            emit_s_matmuls(w, i)
                        elif kind == "e":
                            emit_e_batch(w, i)
                        elif kind == "z":
                            emit_z_chunk(w, bd[i] * SB, bd[i + 1] * SB)
                        elif kind == "w":
                            emit_wsum_pair(w, i)
                        elif kind == "a":
                            emit_anchor_pair(w, i)

                emit_window_setup(0)
                emit_window_setup(1)
                for j in range(NP):
                    emit_anchor_pair(0, j)
                emit_fv(0)
                seq0, tail0 = pass2b(0)
                dispatch(seq0)
                dispatch(_weave2([("a", 1, j) for j in range(NP)], tail0))
                emit_output(0)
                emit_fv(1)
                seq1, tail1 = pass2b(1)
                dispatch(seq1)
                dispatch(tail1)
                emit_output(1)

            if loop_repeat is not None:
                import inspect as _insp
                _kw = {}
                if "staggered_reset" in _insp.signature(tc.For_i).parameters:
                    _kw["staggered_reset"] = _STAGGER
                with tc.For_i(0, loop_repeat, 1, **_kw):
                    for u in range(_UNROLL):
                        body(f"L{u}")
            else:
                for rep in range(repeat):
                    body(rep)

    return nc


def _weave2(a_items, b_items):
    """Proportionally interleave two lists, preserving each list's order."""
    na, nb = len(a_items), len(b_items)
    out = []
    ai = bi = 0
    while ai < na or bi < nb:
        if ai < na and (bi >= nb or ai * nb <= bi * na):
            out.append(a_items[ai]); ai += 1
        else:
            out.append(b_items[bi]); bi += 1
    return out


def _prepare(ifeat, Wu, Wv, bv, we, seg_ids):
    """Host-side shard + pad + layout. Returns (T_W, in_maps)."""
    ifeat = np.asarray(ifeat, dtype=np.float32)
    Wu = np.asarray(Wu, dtype=np.float32)
    Wv = np.asarray(Wv, dtype=np.float32)
    bv = np.asarray(bv, dtype=np.float32)
    we = np.asarray(we, dtype=np.float32)
    seg_ids = np.asarray(seg_ids)

    W = N_WINDOWS
    bounds = np.searchsorted(
        seg_ids, np.arange(0, B + 1, SEGS_PER_WINDOW), side="left")
    n_w = np.diff(bounds)
    T_W = max(4, int(-(-int(n_w.max()) // P)))
    T_W = ((T_W + 3) // 4) * 4
    NT = W_PER_CORE * T_W

    win = (seg_ids // SEGS_PER_WINDOW).astype(np.int64)
    pos = np.arange(N, dtype=np.int64) - bounds[win]
    sloc = (seg_ids % SEGS_PER_WINDOW).astype(np.int64)

    if8 = ifeat.astype(F8)
    # error-diffused fp8 rounding of nat: within each segment (nodes sorted),
    # carry the rounding residual forward so segment sums are nearly exact.
    natq = np.empty((N, D), dtype=F8)
    carry = np.zeros(D, dtype=np.float32)
    seg_np = np.asarray(seg_ids, dtype=np.int64)
    prev = -1
    for i in range(N):
        s = seg_np[i]
        if s != prev:
            carry[:] = 0.0
            prev = s
        v = ifeat[i] + carry
        q = v.astype(F8)
        carry = v - q.astype(np.float32)
        natq[i] = q
    natA = np.zeros((W, T_W * P, D), dtype=F8)
    natA[win, pos, :] = natq
    ifA = np.zeros((W, T_W * P, D), dtype=F8)   # nearest-rounded for fc_u
    ifA[win, pos, :] = if8
    ohwA = np.zeros((W, T_W * P, P), dtype=F8)
    ohwA[win, pos, sloc] = 1.0

    counts = np.bincount(np.asarray(seg_ids, dtype=np.int64), minlength=B)
    rcnt = (1.0 / np.maximum(counts, 1)).astype(np.float32).reshape(W, P, 1)

    wuT8 = (np.ascontiguousarray(Wu.T) * WU_SCALE).reshape(2, P, D)
    wuT8 = np.ascontiguousarray(wuT8.transpose(1, 0, 2)).astype(F8)  # [P,2,D]
    wvT8 = (np.ascontiguousarray(Wv.T) * WU_SCALE).reshape(2, P, D)
    wvT8 = np.ascontiguousarray(wvT8.transpose(1, 0, 2)).astype(F8)
    web = np.ascontiguousarray(we.reshape(2, P).T).astype(BF)  # [dlo, half]
    bvb8 = np.tile(bv * FV_SCALE, (P, 1)).astype(np.float32)
    idb = np.eye(P, dtype=BF)
    one8 = np.ones((P, 2), dtype=F8)

    in_maps = []
    for c in range(N_CORES):
        X = natA[2 * c:2 * c + 2].reshape(W_PER_CORE, T_W, P, D)
        # nat [lane, (w,t), d]
        natp = np.ascontiguousarray(
            X.transpose(2, 0, 1, 3).reshape(P, NT, D))
        # ifT [d_lo, (w,t), kb, lane]
        Y = ifA[2 * c:2 * c + 2].reshape(W_PER_CORE, T_W, P, 2, P)
        iftp = np.ascontiguousarray(
            Y.transpose(4, 0, 1, 3, 2).reshape(P, NT, 2, P))
        O = ohwA[2 * c:2 * c + 2].reshape(W_PER_CORE, T_W, P, P)
        ohwp = np.ascontiguousarray(
            O.transpose(2, 0, 1, 3).reshape(P, NT, P))
        ohtp = np.ascontiguousarray(
            O.transpose(3, 0, 1, 2).reshape(P, NT, P))
        in_maps.append({
            "natp": natp, "iftp": iftp, "ohwp": ohwp, "ohtp": ohtp,
            "wuT8": wuT8, "wvT8": wvT8, "web": web, "bvb8": bvb8,
            "idb": idb, "one8": one8,
            "rcnt": rcnt[2 * c:2 * c + 2],
        })
    return T_W, in_maps


_DEBUG = False
_LAST = {}


def _run(ifeat, Wu, Wv, bv, we, seg_ids, trace=False):
    from concourse.bass_utils import run_bass_kernel_spmd

    T_W, in_maps = _prepare(ifeat, Wu, Wv, bv, we, seg_ids)
    nc = _build(T_W)
    _split_sync_waits(nc)
    res = run_bass_kernel_spmd(nc, in_maps, list(range(N_CORES)), trace=trace)
    _LAST["res"] = res
    _LAST["T_W"] = T_W
    _LAST["nc"] = nc
    _LAST["in_maps"] = in_maps

    out = np.empty((B, 2 * D), dtype=np.float32)
    for c in range(N_CORES):
        core_out = res.results[c]["out"]  # [W_PER_CORE, P, 2D]
        for wl in range(W_PER_CORE):
            w = c * W_PER_CORE + wl
            out[w * SEGS_PER_WINDOW:(w + 1) * SEGS_PER_WINDOW, :] = core_out[wl]
    return out


def kernel(ifeat, Wu, Wv, bv, we, seg_ids):
    return _run(ifeat, Wu, Wv, bv, we, seg_ids, trace=False)


# revision 45
# speedup vs baseline: 2.4939x; 1.1449x over previous
"""Trainium2 Bass kernel for nn_AttnReadout (segment attention readout).

Computation (reference):
    anchor[b]  = mean of ifeat rows in segment b                  [B, D]
    e[i]       = sigmoid(ifeat @ Wu.T + (anchor @ Wv.T + bv)[seg]) @ we
    alpha      = segment_softmax(e)
    rst[b]     = sum_i alpha[i] * ifeat[i]                        [B, D]
    out        = concat([rst, anchor], axis=1)                    [B, 2D]

Sharding: 2048 segments -> 8 cores x 2 windows of 128 contiguous segments.
Nodes (sorted by segment) are padded per-window to T_W tiles of 128 rows.

All heavy matmuls run in fp8e4 with MatmulPerfMode.DoubleRow (2 k-tiles per
instruction, 0.5 cycles per output column):
  - anchor / weighted-sum segment reductions pair ADJACENT node tiles as the
    two k-tiles (one-hot pair as stationary, feature pair as moving).
  - fc_u contracts the two 128-feature halves as k-tiles.
  - the fv gather duplicates its single 128-seg k-tile via stride-0 APs and
    pre-halves fv to compensate.
One-hot matrices in both layouts (ohw [node,seg] and ohT [seg,node]) are
host-prepared from seg_ids and DMAed as fp8 (no on-device transposes).
Sigmoid runs on batches of 4 node tiles from a 2-bank PSUM region to
amortize the activation-engine access latency; z = exp(e) uses the
sigmoid(e)/sigmoid(-e) identity to stay on one ACT table.

Scaling: Wu.T and Wv.T are stored x16 (good fp8 range), anchor means x4,
fv stored as 8*(fv+bv) (halved for the stride-0 duplication), and the
sigmoid applies scale=1/16 to undo it. The z scale cancels in rst/denom.
"""

import numpy as np
import ml_dtypes

N = 102400
D = 256
B = 2048
N_CORES = 8
W_PER_CORE = 2
N_WINDOWS = N_CORES * W_PER_CORE  # 16
SEGS_PER_WINDOW = B // N_WINDOWS  # 128
P = 128
BF = ml_dtypes.bfloat16
F8 = ml_dtypes.float8_e4m3fn

WU_SCALE = 16.0
ANCH_SCALE = 4.0
FV_SCALE = 8.0  # fv stored as FV_SCALE*(fv+bv); doubled by dup -> 16
SB = 4          # sigmoid batch (node tiles per activation)


def _apply_tile_patch():
    """Split TileContext's multi-wait tail drain into single-wait drains
    (this walrus build rejects >1 sync wait on a Drain instruction)."""
    import concourse.tile as tile_mod
    from concourse.vector_clock import ScopedClock

    if getattr(tile_mod.TileContext, "_drain_wait_split_patch", False):
        return

    def _patched(self, tick_clock, wait_clock):
        nc = self.nc
        drain_inst = nc.sync.drain()
        wait_clock.add_sem_waits(
            drain_inst.ins, ScopedClock({None: tick_clock.global_clock})
        )
        si = drain_inst.ins.sync_info
        waits = list(si.on_wait) if si is not None else []
        if len(waits) > 1:
            SyncInfo = type(si)
            drain_inst.ins.sync_info = SyncInfo(
                on_wait=[waits[0]], on_update=list(si.on_update)
            )
            for w in waits[1:]:
                extra = nc.sync.drain()
                extra.ins.sync_info = SyncInfo(on_wait=[w], on_update=[])

        nc.all_engine_barrier()
        assert self.sems is not None
        popped = nc._tile_sem_poison_stack.pop()
        assert popped is self._sem_poison
        nc.clear_and_free_semaphores(list(self.sems.allocated().values()))
        nc.all_engine_barrier()

    tile_mod.TileContext._drain_and_barrier = _patched
    tile_mod.TileContext._drain_wait_split_patch = True


def _split_sync_waits(nc, limit=1):
    """Split >limit sync waits per instruction into preceding single-wait
    EventSemaphore carriers on the same engine (walrus build limit)."""
    import concourse.mybir as mybir

    n_new = 0
    for _, bassbb in nc.bb_map.items():
        insts = bassbb.bb.instructions  # live list
        snapshot = list(insts)
        offset = 0
        for pos, inst in enumerate(snapshot):
            si = getattr(inst, "sync_info", None)
            if si is None:
                continue
            waits = list(si.on_wait)
            if len(waits) <= limit:
                continue
            SyncInfo = type(si)
            inst.sync_info = SyncInfo(
                on_wait=waits[:limit], on_update=list(si.on_update))
            carriers = []
            for w in waits[limit:]:
                c = mybir.InstEventSemaphore(
                    name=f"WSPLIT-{nc.next_id()}", ins=[], outs=[])
                c.engine = inst.engine
                c.sync_info = SyncInfo(on_wait=[w], on_update=[])
                carriers.append(c)
            insts[pos + offset:pos + offset] = carriers
            offset += len(carriers)
            n_new += len(carriers)
    return n_new


_PACK = {}
_STAGGER = False
_UNROLL = 1
_ELAG = 2
_ZB = 1
_DRAIN = 2


def _build(T_W, repeat=1, loop_repeat=None):
    """Build the single-core SPMD Bass program; T_W must be a multiple of 4.

    Uses _PACK["bases"] (canonical per-tile 32-aligned segment-block base,
    identical across cores) and _PACK["width"] set by _prepare: the one-hot
    of tile t only has nonzero columns in [base(t), base(t)+width)."""
    import contextlib
    import concourse.bass as bass
    import concourse.mybir as mybir
    from concourse.tile import TileContext

    _apply_tile_patch()

    f32 = mybir.dt.float32
    bf16 = mybir.dt.bfloat16
    fp8 = mybir.dt.float8e4
    Alu = mybir.AluOpType
    Act = mybir.ActivationFunctionType
    PM = mybir.MatmulPerfMode

    assert T_W % 4 == 0
    CH = T_W // 2            # tiles per DMA chunk (2 chunks per window), even
    NT = W_PER_CORE * T_W
    NB = T_W // SB           # sigmoid batches per window
    NB0 = NB // 2            # batches in z-chunk 0
    TC0 = NB0 * SB           # tiles in z-chunk 0
    NP = T_W // 2            # node-tile pairs per window
    NP0 = TC0 // 2           # pairs fully covered by z-chunk 0

    nc = bass.Bass("TRN2", num_devices=N_CORES)

    nat_dram = nc.dram_tensor("natp", [P, NT, D], fp8, kind="ExternalInput")
    ifT_dram = nc.dram_tensor("iftp", [P, NT, 2, P], fp8, kind="ExternalInput")
    ohw_dram = nc.dram_tensor("ohwp", [P, NT, P], fp8, kind="ExternalInput")
    ohT_dram = nc.dram_tensor("ohtp", [P, NT, P], fp8, kind="ExternalInput")
    wuT_dram = nc.dram_tensor("wuT8", [P, 2, D], fp8, kind="ExternalInput")
    wvT_dram = nc.dram_tensor("wvT8", [P, 2, D], fp8, kind="ExternalInput")
    web_dram = nc.dram_tensor("web", [P, 4], bf16, kind="ExternalInput")
    bvb_dram = nc.dram_tensor("bvb8", [P, D], f32, kind="ExternalInput")
    idb_dram = nc.dram_tensor("idb", [P, P], bf16, kind="ExternalInput")
    one_dram = nc.dram_tensor("one8", [P, 2], fp8, kind="ExternalInput")
    rcnt_dram = nc.dram_tensor("rcnt", [W_PER_CORE, P, 1], f32,
                               kind="ExternalInput")
    out_dram = nc.dram_tensor("out", [W_PER_CORE, P, 2 * D], f32,
                              kind="ExternalOutput")
    dbg_dram = None
    if _DEBUG:
        dbg_dram = nc.dram_tensor("dbg", [W_PER_CORE, P, 2 * T_W], f32,
                                  kind="ExternalOutput")

    with TileContext(nc) as tc:
        with contextlib.ExitStack() as ctx:
            const_pool = ctx.enter_context(tc.tile_pool(name="const", bufs=1))
            data_pool = ctx.enter_context(tc.tile_pool(name="data", bufs=1))
            sbuf_pool = ctx.enter_context(tc.tile_pool(name="sbuf", bufs=1))
            # PSUM: s_ps 2x2 banks + anchor 1 (tile also hosts the e columns
            # and transpose scratch) + wsum 1 + fv/wlo 2 (cross-rep overlap:
            # next rep's fv must not wait on this rep's final output) = 8.
            anchor_ps_pool = ctx.enter_context(
                tc.tile_pool(name="anchor_ps", bufs=1, space="PSUM"))
            wsum_ps_pool = ctx.enter_context(
                tc.tile_pool(name="wsum_ps", bufs=1, space="PSUM"))
            s_ps_pool = ctx.enter_context(
                tc.tile_pool(name="s_ps", bufs=2, space="PSUM"))
            fv_ps_pool = ctx.enter_context(
                tc.tile_pool(name="fv_ps", bufs=2, space="PSUM"))

            # ---- constants ----
            wuT_sb = const_pool.tile([P, 2, D], fp8, name="wuT_sb", tag="wuT_sb")
            nc.sync.dma_start(wuT_sb[:], wuT_dram[:])
            wvT_sb = const_pool.tile([P, 2, D], fp8, name="wvT_sb", tag="wvT_sb")
            nc.sync.dma_start(wvT_sb[:], wvT_dram[:])
            web_sb = const_pool.tile([P, 4], bf16, name="web_sb", tag="web_sb")
            nc.sync.dma_start(web_sb[:], web_dram[:])
            bvb_sb = const_pool.tile([P, D], f32, name="bvb_sb", tag="bvb_sb")
            nc.sync.dma_start(bvb_sb[:], bvb_dram[:])
            idb_sb = const_pool.tile([P, P], bf16, name="idb_sb", tag="idb_sb")
            nc.sync.dma_start(idb_sb[:], idb_dram[:])
            one_sb = const_pool.tile([P, 2], fp8, name="one_sb", tag="one_sb")
            nc.sync.dma_start(one_sb[:], one_dram[:])
            rcnt_sb = const_pool.tile([P, W_PER_CORE, 1], f32, name="rcnt_sb",
                                      tag="rcnt_sb")
            nc.sync.dma_start(
                rcnt_sb[:], rcnt_dram[:].rearrange("w p one -> p w one"))

            def body(rep):
                # window data, 2 chunks per window, all resident.
                # DMA emission follows consumption order: the anchor pass of a
                # window needs ohw+nat of both its chunks first; ifT/ohT feed
                # the later s-pipeline.
                nat_ch, ifT_ch, ohw_ch, ohT_ch = {}, {}, {}, {}

                def dma_ohw_nat(c):
                    ohwc = data_pool.tile([P, CH, P], fp8,
                                          name=f"ohwc{rep}_{c}", tag="ohwc",
                                          bufs=4)
                    nc.sync.dma_start(ohwc[:], ohw_dram[:, c * CH:(c + 1) * CH, :])
                    ohw_ch[c] = ohwc
                    natc = data_pool.tile([P, CH, D], fp8,
                                          name=f"natc{rep}_{c}", tag="natc",
                                          bufs=4)
                    nc.sync.dma_start(natc[:], nat_dram[:, c * CH:(c + 1) * CH, :])
                    nat_ch[c] = natc

                def dma_ift_oht(c):
                    iftc = data_pool.tile([P, CH, 2, P], fp8,
                                          name=f"iftc{rep}_{c}", tag="iftc",
                                          bufs=4)
                    nc.sync.dma_start(iftc[:], ifT_dram[:, c * CH:(c + 1) * CH, :, :])
                    ifT_ch[c] = iftc
                    ohtc = data_pool.tile([P, CH, P], fp8,
                                          name=f"ohtc{rep}_{c}", tag="ohtc",
                                          bufs=4)
                    nc.sync.dma_start(ohtc[:], ohT_dram[:, c * CH:(c + 1) * CH, :])
                    ohT_ch[c] = ohtc

                dma_ohw_nat(0)
                dma_ohw_nat(1)
                dma_ift_oht(0)
                dma_ift_oht(1)
                dma_ohw_nat(2)
                dma_ohw_nat(3)
                dma_ift_oht(2)
                dma_ift_oht(3)

                def nat_pair(w, j):      # [P, 2, D] node-tile pair
                    g = w * T_W + 2 * j
                    return nat_ch[g // CH][:, (g % CH):(g % CH) + 2, :]

                def ohw_pair(w, j):      # [P, 2, P]
                    g = w * T_W + 2 * j
                    return ohw_ch[g // CH][:, (g % CH):(g % CH) + 2, :]

                def ifT_t(w, t):         # [P, 2, P]
                    g = w * T_W + t
                    return ifT_ch[g // CH][:, g % CH, :, :]

                def ohT_t(w, t):         # [P, P]
                    g = w * T_W + t
                    return ohT_ch[g // CH][:, g % CH, :]

                def ohw_t(w, t):         # [P, P]
                    g = w * T_W + t
                    return ohw_ch[g // CH][:, g % CH, :]

                # per-window state
                st = {}

                def emit_anchor_pair(w, j):
                    nc.tensor.matmul(st[w]["anchor_ps"][:, 0:D], ohw_pair(w, j),
                                     nat_pair(w, j), start=(j == 0),
                                     stop=(j == NP - 1), perf_mode=PM.DoubleRow)

                def emit_window_setup(w):
                    # anchor bank also hosts the per-batch +e/-e columns
                    # [D : D+2*T_W] and two bf16 transpose scratch regions
                    anchor_ps = anchor_ps_pool.tile(
                        [P, D + 2 * T_W + P], f32, name=f"anc{rep}_{w}",
                        tag="anchor_ps")
                    out_sb = sbuf_pool.tile([P, 2 * D], f32, name=f"osb{rep}_{w}",
                                            tag="out_sb", bufs=2)
                    e_win = sbuf_pool.tile([P, T_W], f32, name=f"ew{rep}_{w}",
                                           tag="e_win", bufs=2)
                    z_win = sbuf_pool.tile([P, T_W], f32, name=f"zw{rep}_{w}",
                                           tag="z_win", bufs=2)
                    z8_win = sbuf_pool.tile([P, T_W], f32, name=f"z8{rep}_{w}",
                                            tag="z8_win", bufs=2)
                    zlo_win = sbuf_pool.tile([P, T_W], f32, name=f"zl{rep}_{w}",
                                             tag="zlo_win", bufs=2)
                    wsum_ps = wsum_ps_pool.tile([P, D + 1], f32,
                                                name=f"ws{rep}_{w}", tag="wsum_ps")
                    st[w] = dict(anchor_ps=anchor_ps, out_sb=out_sb,
                                 e_win=e_win, z_win=z_win, z8_win=z8_win,
                                 zlo_win=zlo_win, wsum_ps=wsum_ps)

                def emit_fv(w):
                    anchor_ps = st[w]["anchor_ps"]
                    rc = rcnt_sb[:, w, :]
                    # anchor output half (Act engine keeps DVE free; reads PSUM)
                    nc.scalar.mul(st[w]["out_sb"][:, D:2 * D], anchor_ps[:, 0:D],
                                  rc)
                    anchb = sbuf_pool.tile([P, D], bf16, name=f"a8{rep}_{w}",
                                           tag="anchb", bufs=1)
                    nc.vector.tensor_scalar(anchb[:], anchor_ps[:, 0:D], rc,
                                            ANCH_SCALE, Alu.mult, Alu.mult)
                    anchT = sbuf_pool.tile([P, 2, P], fp8, name=f"aT{rep}_{w}",
                                           tag="anchT", bufs=1)
                    trbase = D + 2 * T_W
                    for db in range(2):
                        trp = (anchor_ps[:, trbase + db * (P // 2):
                                         trbase + (db + 1) * (P // 2)]
                               .bitcast(bf16))
                        nc.tensor.transpose(trp, anchb[:, db * P:(db + 1) * P],
                                            idb_sb[:])
                        nc.vector.tensor_copy(anchT[:, db, :], trp)
                    fv_ps = fv_ps_pool.tile([P, D], f32, name=f"fvp{rep}_{w}",
                                            tag="fv_ps")
                    nc.tensor.matmul(fv_ps[:], anchT[:], wvT_sb[:],
                                     start=True, stop=True, perf_mode=PM.DoubleRow)
                    fv8 = sbuf_pool.tile([P, D], fp8, name=f"fv8{rep}_{w}",
                                         tag="fv8", bufs=1)
                    # fv8 = fv_ps * (FV_SCALE / (WU_SCALE*ANCH_SCALE)) + bv*FV_SCALE
                    nc.vector.scalar_tensor_tensor(
                        out=fv8[:], in0=fv_ps[:],
                        scalar=FV_SCALE / (WU_SCALE * ANCH_SCALE), in1=bvb_sb[:],
                        op0=Alu.mult, op1=Alu.add)
                    st[w]["fv8"] = fv8

                def emit_s_matmuls(w, b):
                    # transposed s: sT[do_half, node] per (half, q); the e
                    # reduction is then a ~free stationary-sT matmul with the
                    # we column as the 1-wide moving operand.
                    s_ps = s_ps_pool.tile([P, 2, SB, P], f32,
                                          name=f"sp{rep}_{w}_{b}", tag="s_ps")
                    fv8 = st[w]["fv8"]
                    for q in range(SB):
                        t = b * SB + q
                        oht_dup = ohT_t(w, t).unsqueeze(1).broadcast_to([P, 2, P])
                        for h in range(2):
                            nc.tensor.matmul(s_ps[:, h, q, :],
                                             wuT_sb[:, :, h * P:(h + 1) * P],
                                             ifT_t(w, t), start=True, stop=False,
                                             perf_mode=PM.DoubleRow)
                            fv_dup = (fv8[:, h * P:(h + 1) * P]
                                      .unsqueeze(1).broadcast_to([P, 2, P]))
                            nc.tensor.matmul(s_ps[:, h, q, :], fv_dup, oht_dup,
                                             start=False, stop=True,
                                             perf_mode=PM.DoubleRow)
                    s_sb = sbuf_pool.tile([P, 2, SB, P], bf16,
                                          name=f"ss{rep}_{w}_{b}", tag="s_sb",
                                          bufs=_ELAG + 1)
                    nc.scalar.activation(
                        s_sb[:].rearrange("p h q d -> p (h q d)"),
                        s_ps[:].rearrange("p h q d -> p (h q d)"),
                        Act.Sigmoid, scale=1.0 / WU_SCALE)
                    st[w][f"s_sb{b}"] = s_sb

                def emit_e_batch(w, b):
                    # writes +e and -e columns (negated we consts) so the z
                    # sigmoid pass needs a single activation per chunk
                    anchor_ps = st[w]["anchor_ps"]
                    s_sb = st[w].pop(f"s_sb{b}")
                    for q in range(SB):
                        for sgn in range(2):
                            ec = D + 8 * b + 4 * sgn + q
                            for h in range(2):
                                nc.tensor.matmul(anchor_ps[:, ec:ec + 1],
                                                 s_sb[:, h, q, :],
                                                 web_sb[:, 2 * sgn + h:
                                                        2 * sgn + h + 1],
                                                 start=(h == 0), stop=(h == 1))
                    if _DEBUG:
                        nc.vector.tensor_copy(
                            st[w]["e_win"][:, b * SB:(b + 1) * SB],
                            anchor_ps[:, D + 8 * b:D + 8 * b + SB])

                def emit_z_chunk(w, b0, b1):
                    # one activation covers the interleaved [+e(4) -e(4)] cols
                    anchor_ps, z_win = st[w]["anchor_ps"], st[w]["z_win"]
                    nb = b1 - b0
                    spn = sbuf_pool.tile([P, nb, 2, SB], f32,
                                         name=f"zp{rep}_{w}_{b0}", tag="zch",
                                         bufs=4)
                    nc.scalar.activation(
                        spn[:].rearrange("p b s q -> p (b s q)"),
                        anchor_ps[:, D + 8 * b0:D + 8 * b1], Act.Sigmoid)
                    rn = sbuf_pool.tile([P, nb, SB], f32,
                                        name=f"zr{rep}_{w}_{b0}", tag="zrn",
                                        bufs=4)
                    nc.vector.reciprocal(rn[:], spn[:, :, 1, :])
                    nc.vector.tensor_tensor(
                        z_win[:, b0 * SB:b1 * SB].rearrange(
                            "p (b q) -> p b q", b=nb),
                        spn[:, :, 0, :], rn[:], Alu.mult)
                    c0, c1 = b0 * SB, b1 * SB
                    # two-term z: z8 = fp8-rounded z (kept in f32 so the scalar
                    # multiply with exact one-hots stores exactly in fp8),
                    # zlo = z - z8 (fp8-stored residual)
                    z8_win, zlo_win = st[w]["z8_win"], st[w]["zlo_win"]
                    z8q = sbuf_pool.tile([P, c1 - c0], fp8,
                                         name=f"zq{rep}_{w}_{c0}",
                                         tag="z8q", bufs=4)
                    nc.vector.tensor_copy(z8q[:], z_win[:, c0:c1])
                    nc.vector.tensor_copy(z8_win[:, c0:c1], z8q[:])
                    nc.vector.tensor_tensor(zlo_win[:, c0:c1], z_win[:, c0:c1],
                                            z8_win[:, c0:c1], Alu.subtract)

                def emit_wsum_pair(w, j):
                    wsum_ps = st[w]["wsum_ps"]
                    z8_win, zlo_win = st[w]["z8_win"], st[w]["zlo_win"]
                    if j == 0:
                        # wsum_lo reuses the fv bank (same pool tag, bufs=1)
                        st[w]["wlo_ps"] = fv_ps_pool.tile(
                            [P, D], f32, name=f"wlo{rep}_{w}", tag="fv_ps")
                    wlo_ps = st[w]["wlo_ps"]
                    ohz = sbuf_pool.tile([P, 2, P], fp8, name=f"oz{rep}_{w}_{j}",
                                         tag="ohz", bufs=4)
                    ohzlo = sbuf_pool.tile([P, 2, P], fp8, name=f"ol{rep}_{w}_{j}",
                                           tag="ohzlo", bufs=4)
                    for k in range(2):
                        t = 2 * j + k
                        nc.vector.tensor_scalar(ohz[:, k, :], ohw_t(w, t),
                                                z8_win[:, t:t + 1], None, Alu.mult)
                        nc.vector.tensor_scalar(ohzlo[:, k, :], ohw_t(w, t),
                                                zlo_win[:, t:t + 1], None,
                                                Alu.mult)
                    nc.tensor.matmul(wsum_ps[:, 0:D], ohz[:], nat_pair(w, j),
                                     start=(j == 0), stop=(j == NP - 1),
                                     perf_mode=PM.DoubleRow)
                    # NOTE: further accumulation groups in the same PSUM bank must
                    # ride the first group's start/stop envelope (start=False;
                    # the j==0 start above zeroes the whole bank) — interleaved
                    # groups with their own start wipe the bank's other region.
                    nc.tensor.matmul(wsum_ps[:, D:D + 1], ohz[:],
                                     one_sb[:].unsqueeze(2),
                                     start=False, stop=False,
                                     perf_mode=PM.DoubleRow,
                                     skip_group_check=True)
                    nc.tensor.matmul(wsum_ps[:, D:D + 1], ohzlo[:],
                                     one_sb[:].unsqueeze(2),
                                     start=False, stop=(j == NP - 1),
                                     perf_mode=PM.DoubleRow,
                                     skip_group_check=True)
                    nc.tensor.matmul(wlo_ps[:], ohzlo[:], nat_pair(w, j),
                                     start=(j == 0), stop=(j == NP - 1),
                                     perf_mode=PM.DoubleRow)

                def emit_output(w):
                    wsum_ps, out_sb = st[w]["wsum_ps"], st[w]["out_sb"]
                    wlo_ps = st[w]["wlo_ps"]
                    den = sbuf_pool.tile([P, 1], f32, name=f"dn{rep}_{w}",
                                         tag="den", bufs=2)
                    nc.vector.tensor_scalar(den[:], wsum_ps[:, D:D + 1], 1e-30,
                                            None, Alu.max)
                    rden = sbuf_pool.tile([P, 1], f32, name=f"rd{rep}_{w}",
                                          tag="rden", bufs=2)
                    nc.vector.reciprocal(rden[:], den[:])
                    wlo_sc = sbuf_pool.tile([P, D], f32, name=f"wl{rep}_{w}",
                                            tag="wlo_sc", bufs=2)
                    nc.scalar.mul(wlo_sc[:], wlo_ps[:], rden[:])
                    nc.vector.scalar_tensor_tensor(
                        out=out_sb[:, 0:D], in0=wsum_ps[:, 0:D], scalar=rden[:],
                        in1=wlo_sc[:], op0=Alu.mult, op1=Alu.add)
                    nc.sync.dma_start(out_dram[w], out_sb[:])
                    if _DEBUG:
                        dbg = sbuf_pool.tile([P, 2 * T_W], f32,
                                             name=f"dbg{rep}_{w}", tag="dbg",
                                             bufs=2)
                        nc.vector.tensor_copy(dbg[:, 0:T_W], st[w]["e_win"][:])
                        nc.vector.tensor_copy(dbg[:, T_W:], st[w]["z_win"][:])
                        nc.sync.dma_start(dbg_dram[w], dbg[:])

                # ---------------- emission schedule ----------------
                # Software-pipelined: e-matmuls lag their sigmoid batch by
                # E_LAG so the in-order PE stream never stalls on Act; z runs
                # in fine-grained chunks (ZB batches) so wsum pairs drain
                # incrementally; the small leftover tail of window 0 overlaps
                # window 1's anchor pass (anchor banks alternate by parity).
                E_LAG = _ELAG
                ZB = _ZB
                bd = list(range(0, NB, ZB)) + [NB]
                if bd[-2] == NB:
                    bd.pop()

                def pass2b(w):
                    from collections import deque
                    seq, ready = [], deque()

                    def drain(n):
                        for _ in range(n):
                            if ready:
                                seq.append(ready.popleft())

                    def after_e(eb):
                        seq.append(("e", w, eb))
                        drain(_DRAIN)
                        if (eb + 1) in bd[1:]:
                            c = bd.index(eb + 1) - 1
                            seq.append(("z", w, c))
                            for j in range(bd[c] * 2, bd[c + 1] * 2):
                                ready.append(("w", w, j))

                    for b in range(NB):
                        seq.append(("s", w, b))
                        drain(_DRAIN)
                        if b - E_LAG >= 0:
                            after_e(b - E_LAG)
                    for eb in range(max(0, NB - E_LAG), NB):
                        after_e(eb)
                    return seq, list(ready)

                def dispatch(seq):
                    for kind, w, i in seq:
                        if kind == "s":
                            emit_s_matmuls(w, i)
                        elif kind == "e":
                            emit_e_batch(w, i)
                        elif kind == "z":
                            emit_z_chunk(w, bd[i], bd[i + 1])
                        elif kind == "w":
                            emit_wsum_pair(w, i)
                        elif kind == "a":
                            emit_anchor_pair(w, i)

                emit_window_setup(0)
                emit_window_setup(1)
                for j in range(NP):
                    emit_anchor_pair(0, j)
                emit_fv(0)
                seq0, tail0 = pass2b(0)
                dispatch(seq0)
                dispatch(_weave2([("a", 1, j) for j in range(NP)], tail0))
                emit_output(0)
                emit_fv(1)
                seq1, tail1 = pass2b(1)
                dispatch(seq1)
                dispatch(tail1)
                emit_output(1)

            if loop_repeat is not None:
                import inspect as _insp
                _kw = {}
                if "staggered_reset" in _insp.signature(tc.For_i).parameters:
                    _kw["staggered_reset"] = _STAGGER
                with tc.For_i(0, loop_repeat, 1, **_kw):
                    for u in range(_UNROLL):
                        body(f"L{u}")
            else:
                for rep in range(repeat):
                    body(rep)

    return nc


def _weave2(a_items, b_items):
    """Proportionally interleave two lists, preserving each list's order."""
    na, nb = len(a_items), len(b_items)
    out = []
    ai = bi = 0
    while ai < na or bi < nb:
        if ai < na and (bi >= nb or ai * nb <= bi * na):
            out.append(a_items[ai]); ai += 1
        else:
            out.append(b_items[bi]); bi += 1
    return out


def _prepare(ifeat, Wu, Wv, bv, we, seg_ids):
    """Host-side shard + pad + layout. Returns (T_W, in_maps)."""
    ifeat = np.asarray(ifeat, dtype=np.float32)
    Wu = np.asarray(Wu, dtype=np.float32)
    Wv = np.asarray(Wv, dtype=np.float32)
    bv = np.asarray(bv, dtype=np.float32)
    we = np.asarray(we, dtype=np.float32)
    seg_ids = np.asarray(seg_ids)

    W = N_WINDOWS
    bounds = np.searchsorted(
        seg_ids, np.arange(0, B + 1, SEGS_PER_WINDOW), side="left")
    n_w = np.diff(bounds)
    T_W = max(4, int(-(-int(n_w.max()) // P)))
    T_W = ((T_W + 3) // 4) * 4
    NT = W_PER_CORE * T_W

    win = (seg_ids // SEGS_PER_WINDOW).astype(np.int64)
    pos = np.arange(N, dtype=np.int64) - bounds[win]
    sloc = (seg_ids % SEGS_PER_WINDOW).astype(np.int64)

    if8 = ifeat.astype(F8)
    # error-diffused fp8 rounding of nat: within each segment (nodes sorted),
    # carry the rounding residual forward so segment sums are nearly exact.
    natq = np.empty((N, D), dtype=F8)
    carry = np.zeros(D, dtype=np.float32)
    seg_np = np.asarray(seg_ids, dtype=np.int64)
    prev = -1
    for i in range(N):
        s = seg_np[i]
        if s != prev:
            carry[:] = 0.0
            prev = s
        v = ifeat[i] + carry
        q = v.astype(F8)
        carry = v - q.astype(np.float32)
        natq[i] = q
    natA = np.zeros((W, T_W * P, D), dtype=F8)
    natA[win, pos, :] = natq
    ifA = np.zeros((W, T_W * P, D), dtype=F8)   # nearest-rounded for fc_u
    ifA[win, pos, :] = if8
    ohwA = np.zeros((W, T_W * P, P), dtype=F8)
    ohwA[win, pos, sloc] = 1.0

    counts = np.bincount(np.asarray(seg_ids, dtype=np.int64), minlength=B)
    rcnt = (1.0 / np.maximum(counts, 1)).astype(np.float32).reshape(W, P, 1)

    wuT8 = (np.ascontiguousarray(Wu.T) * WU_SCALE).reshape(2, P, D)
    wuT8 = np.ascontiguousarray(wuT8.transpose(1, 0, 2)).astype(F8)  # [P,2,D]
    wvT8 = (np.ascontiguousarray(Wv.T) * WU_SCALE).reshape(2, P, D)
    wvT8 = np.ascontiguousarray(wvT8.transpose(1, 0, 2)).astype(F8)
    web = np.concatenate([we.reshape(2, P).T, -we.reshape(2, P).T],
                         axis=1).astype(BF)  # [dlo, (+h0,+h1,-h0,-h1)]
    bvb8 = np.tile(bv * FV_SCALE, (P, 1)).astype(np.float32)
    idb = np.eye(P, dtype=BF)
    one8 = np.ones((P, 2), dtype=F8)

    in_maps = []
    for c in range(N_CORES):
        X = natA[2 * c:2 * c + 2].reshape(W_PER_CORE, T_W, P, D)
        # nat [lane, (w,t), d]
        natp = np.ascontiguousarray(
            X.transpose(2, 0, 1, 3).reshape(P, NT, D))
        # ifT [d_lo, (w,t), kb, lane]
        Y = ifA[2 * c:2 * c + 2].reshape(W_PER_CORE, T_W, P, 2, P)
        iftp = np.ascontiguousarray(
            Y.transpose(4, 0, 1, 3, 2).reshape(P, NT, 2, P))
        O = ohwA[2 * c:2 * c + 2].reshape(W_PER_CORE, T_W, P, P)
        ohwp = np.ascontiguousarray(
            O.transpose(2, 0, 1, 3).reshape(P, NT, P))
        ohtp = np.ascontiguousarray(
            O.transpose(3, 0, 1, 2).reshape(P, NT, P))
        in_maps.append({
            "natp": natp, "iftp": iftp, "ohwp": ohwp, "ohtp": ohtp,
            "wuT8": wuT8, "wvT8": wvT8, "web": web, "bvb8": bvb8,
            "idb": idb, "one8": one8,
            "rcnt": rcnt[2 * c:2 * c + 2],
        })
    return T_W, in_maps


_DEBUG = False
_LAST = {}


def _run(ifeat, Wu, Wv, bv, we, seg_ids, trace=False):
    from concourse.bass_utils import run_bass_kernel_spmd

    T_W, in_maps = _prepare(ifeat, Wu, Wv, bv, we, seg_ids)
    nc = _build(T_W)
    _split_sync_waits(nc)
    res = run_bass_kernel_spmd(nc, in_maps, list(range(N_CORES)), trace=trace)
    _LAST["res"] = res
    _LAST["T_W"] = T_W
    _LAST["nc"] = nc
    _LAST["in_maps"] = in_maps

    out = np.empty((B, 2 * D), dtype=np.float32)
    for c in range(N_CORES):
        core_out = res.results[c]["out"]  # [W_PER_CORE, P, 2D]
        for wl in range(W_PER_CORE):
            w = c * W_PER_CORE + wl
            out[w * SEGS_PER_WINDOW:(w + 1) * SEGS_PER_WINDOW, :] = core_out[wl]
    return out


def kernel(ifeat, Wu, Wv, bv, we, seg_ids):
    return _run(ifeat, Wu, Wv, bv, we, seg_ids, trace=False)


# revision 49
# speedup vs baseline: 2.7555x; 1.1049x over previous
"""Trainium2 Bass kernel for nn_AttnReadout (segment attention readout).

Computation (reference):
    anchor[b]  = mean of ifeat rows in segment b                  [B, D]
    e[i]       = sigmoid(ifeat @ Wu.T + (anchor @ Wv.T + bv)[seg]) @ we
    alpha      = segment_softmax(e)
    rst[b]     = sum_i alpha[i] * ifeat[i]                        [B, D]
    out        = concat([rst, anchor], axis=1)                    [B, 2D]

Sharding: 2048 segments -> 8 cores x 2 windows of 128 contiguous segments.
Nodes (sorted by segment) are padded per-window to T_W tiles of 128 rows.

All heavy matmuls run in fp8e4 with MatmulPerfMode.DoubleRow (2 k-tiles per
instruction, 0.5 cycles per output column):
  - anchor / weighted-sum segment reductions pair ADJACENT node tiles as the
    two k-tiles (one-hot pair as stationary, feature pair as moving).
  - fc_u contracts the two 128-feature halves as k-tiles.
  - the fv gather duplicates its single 128-seg k-tile via stride-0 APs and
    pre-halves fv to compensate.
One-hot matrices in both layouts (ohw [node,seg] and ohT [seg,node]) are
host-prepared from seg_ids and DMAed as fp8 (no on-device transposes).
Sigmoid runs on batches of 4 node tiles from a 2-bank PSUM region to
amortize the activation-engine access latency; z = exp(e) uses the
sigmoid(e)/sigmoid(-e) identity to stay on one ACT table.

Scaling: Wu.T and Wv.T are stored x16 (good fp8 range), anchor means x4,
fv stored as 8*(fv+bv) (halved for the stride-0 duplication), and the
sigmoid applies scale=1/16 to undo it. The z scale cancels in rst/denom.
"""

import numpy as np
import ml_dtypes

N = 102400
D = 256
B = 2048
N_CORES = 8
W_PER_CORE = 2
N_WINDOWS = N_CORES * W_PER_CORE  # 16
SEGS_PER_WINDOW = B // N_WINDOWS  # 128
P = 128
BF = ml_dtypes.bfloat16
F8 = ml_dtypes.float8_e4m3fn

WU_SCALE = 16.0
ANCH_SCALE = 4.0
FV_SCALE = 8.0  # fv stored as FV_SCALE*(fv+bv); doubled by dup -> 16
SB = 4          # sigmoid batch (node tiles per activation)


def _apply_tile_patch():
    """Split TileContext's multi-wait tail drain into single-wait drains
    (this walrus build rejects >1 sync wait on a Drain instruction)."""
    import concourse.tile as tile_mod
    from concourse.vector_clock import ScopedClock

    if getattr(tile_mod.TileContext, "_drain_wait_split_patch", False):
        return

    def _patched(self, tick_clock, wait_clock):
        nc = self.nc
        drain_inst = nc.sync.drain()
        wait_clock.add_sem_waits(
            drain_inst.ins, ScopedClock({None: tick_clock.global_clock})
        )
        si = drain_inst.ins.sync_info
        waits = list(si.on_wait) if si is not None else []
        if len(waits) > 1:
            SyncInfo = type(si)
            drain_inst.ins.sync_info = SyncInfo(
                on_wait=[waits[0]], on_update=list(si.on_update)
            )
            for w in waits[1:]:
                extra = nc.sync.drain()
                extra.ins.sync_info = SyncInfo(on_wait=[w], on_update=[])

        nc.all_engine_barrier()
        assert self.sems is not None
        popped = nc._tile_sem_poison_stack.pop()
        assert popped is self._sem_poison
        nc.clear_and_free_semaphores(list(self.sems.allocated().values()))
        nc.all_engine_barrier()

    tile_mod.TileContext._drain_and_barrier = _patched
    tile_mod.TileContext._drain_wait_split_patch = True


def _split_sync_waits(nc, limit=1):
    """Split >limit sync waits per instruction into preceding single-wait
    EventSemaphore carriers on the same engine (walrus build limit)."""
    import concourse.mybir as mybir

    n_new = 0
    for _, bassbb in nc.bb_map.items():
        insts = bassbb.bb.instructions  # live list
        snapshot = list(insts)
        offset = 0
        for pos, inst in enumerate(snapshot):
            si = getattr(inst, "sync_info", None)
            if si is None:
                continue
            waits = list(si.on_wait)
            if len(waits) <= limit:
                continue
            SyncInfo = type(si)
            inst.sync_info = SyncInfo(
                on_wait=waits[:limit], on_update=list(si.on_update))
            carriers = []
            for w in waits[limit:]:
                c = mybir.InstEventSemaphore(
                    name=f"WSPLIT-{nc.next_id()}", ins=[], outs=[])
                c.engine = inst.engine
                c.sync_info = SyncInfo(on_wait=[w], on_update=[])
                carriers.append(c)
            insts[pos + offset:pos + offset] = carriers
            offset += len(carriers)
            n_new += len(carriers)
    return n_new


_PACK = {}
_STAGGER = False
_UNROLL = 1
_ELAG = 2
_ZB = 1
_DRAIN = 2


def _build(T_W, repeat=1, loop_repeat=None):
    """Build the single-core SPMD Bass program; T_W must be a multiple of 4.

    Uses _PACK["bases"] (canonical per-tile 32-aligned segment-block base,
    identical across cores) and _PACK["width"] set by _prepare: the one-hot
    of tile t only has nonzero columns in [base(t), base(t)+width)."""
    import contextlib
    import concourse.bass as bass
    import concourse.mybir as mybir
    from concourse.tile import TileContext

    _apply_tile_patch()

    f32 = mybir.dt.float32
    bf16 = mybir.dt.bfloat16
    fp8 = mybir.dt.float8e4
    Alu = mybir.AluOpType
    Act = mybir.ActivationFunctionType
    PM = mybir.MatmulPerfMode

    assert T_W % 4 == 0
    CH = T_W // 2            # tiles per DMA chunk (2 chunks per window), even
    NT = W_PER_CORE * T_W
    NB = T_W // SB           # sigmoid batches per window
    NB0 = NB // 2            # batches in z-chunk 0
    TC0 = NB0 * SB           # tiles in z-chunk 0
    NP = T_W // 2            # node-tile pairs per window
    NP0 = TC0 // 2           # pairs fully covered by z-chunk 0

    nc = bass.Bass("TRN2", num_devices=N_CORES)

    nat_dram = nc.dram_tensor("natp", [P, NT, D], fp8, kind="ExternalInput")
    ifT_dram = nc.dram_tensor("iftp", [P, NT, 2, P], fp8, kind="ExternalInput")
    ohw_dram = nc.dram_tensor("ohwp", [P, NT, P], fp8, kind="ExternalInput")
    ohT_dram = nc.dram_tensor("ohtp", [P, NT, P], fp8, kind="ExternalInput")
    wuT_dram = nc.dram_tensor("wuT8", [P, 2, D], fp8, kind="ExternalInput")
    wvT_dram = nc.dram_tensor("wvT8", [P, 2, D], fp8, kind="ExternalInput")
    web_dram = nc.dram_tensor("web", [P, 4], bf16, kind="ExternalInput")
    bvb_dram = nc.dram_tensor("bvb8", [P, D], f32, kind="ExternalInput")
    idb_dram = nc.dram_tensor("idb", [P, P], bf16, kind="ExternalInput")
    one_dram = nc.dram_tensor("one8", [P, 2], fp8, kind="ExternalInput")
    rcnt_dram = nc.dram_tensor("rcnt", [W_PER_CORE, P, 1], f32,
                               kind="ExternalInput")
    out_dram = nc.dram_tensor("out", [W_PER_CORE, P, 2 * D], f32,
                              kind="ExternalOutput")
    dbg_dram = None
    if _DEBUG:
        dbg_dram = nc.dram_tensor("dbg", [W_PER_CORE, P, 2 * T_W], f32,
                                  kind="ExternalOutput")

    with TileContext(nc) as tc:
        with contextlib.ExitStack() as ctx:
            const_pool = ctx.enter_context(tc.tile_pool(name="const", bufs=1))
            data_pool = ctx.enter_context(tc.tile_pool(name="data", bufs=1))
            sbuf_pool = ctx.enter_context(tc.tile_pool(name="sbuf", bufs=1))
            # PSUM: s_ps 2x2 banks + anchor 1 (tile also hosts the e columns
            # and transpose scratch) + wsum 1 + fv/wlo 2 (cross-rep overlap:
            # next rep's fv must not wait on this rep's final output) = 8.
            anchor_ps_pool = ctx.enter_context(
                tc.tile_pool(name="anchor_ps", bufs=1, space="PSUM"))
            wsum_ps_pool = ctx.enter_context(
                tc.tile_pool(name="wsum_ps", bufs=1, space="PSUM"))
            s_ps_pool = ctx.enter_context(
                tc.tile_pool(name="s_ps", bufs=2, space="PSUM"))
            fv_ps_pool = ctx.enter_context(
                tc.tile_pool(name="fv_ps", bufs=2, space="PSUM"))

            # ---- constants ----
            wuT_sb = const_pool.tile([P, 2, D], fp8, name="wuT_sb", tag="wuT_sb")
            nc.sync.dma_start(wuT_sb[:], wuT_dram[:])
            wvT_sb = const_pool.tile([P, 2, D], fp8, name="wvT_sb", tag="wvT_sb")
            nc.sync.dma_start(wvT_sb[:], wvT_dram[:])
            web_sb = const_pool.tile([P, 4], bf16, name="web_sb", tag="web_sb")
            nc.sync.dma_start(web_sb[:], web_dram[:])
            bvb_sb = const_pool.tile([P, D], f32, name="bvb_sb", tag="bvb_sb")
            nc.sync.dma_start(bvb_sb[:], bvb_dram[:])
            idb_sb = const_pool.tile([P, P], bf16, name="idb_sb", tag="idb_sb")
            nc.sync.dma_start(idb_sb[:], idb_dram[:])
            one_sb = const_pool.tile([P, 2], fp8, name="one_sb", tag="one_sb")
            nc.sync.dma_start(one_sb[:], one_dram[:])
            rcnt_sb = const_pool.tile([P, W_PER_CORE, 1], f32, name="rcnt_sb",
                                      tag="rcnt_sb")
            nc.sync.dma_start(
                rcnt_sb[:], rcnt_dram[:].rearrange("w p one -> p w one"))

            def body(rep):
                # window data, 2 chunks per window, all resident.
                # DMA emission follows consumption order: the anchor pass of a
                # window needs ohw+nat of both its chunks first; ifT/ohT feed
                # the later s-pipeline.
                nat_ch, ifT_ch, ohw_ch, ohT_ch = {}, {}, {}, {}

                def dma_ohw_nat(c):
                    ohwc = data_pool.tile([P, CH, P], fp8,
                                          name=f"ohwc{rep}_{c}", tag="ohwc",
                                          bufs=4)
                    nc.sync.dma_start(ohwc[:], ohw_dram[:, c * CH:(c + 1) * CH, :])
                    ohw_ch[c] = ohwc
                    natc = data_pool.tile([P, CH, D], fp8,
                                          name=f"natc{rep}_{c}", tag="natc",
                                          bufs=4)
                    nc.sync.dma_start(natc[:], nat_dram[:, c * CH:(c + 1) * CH, :])
                    nat_ch[c] = natc

                def dma_ift_oht(c):
                    iftc = data_pool.tile([P, CH, 2, P], fp8,
                                          name=f"iftc{rep}_{c}", tag="iftc",
                                          bufs=4)
                    nc.sync.dma_start(iftc[:], ifT_dram[:, c * CH:(c + 1) * CH, :, :])
                    ifT_ch[c] = iftc
                    ohtc = data_pool.tile([P, CH, P], fp8,
                                          name=f"ohtc{rep}_{c}", tag="ohtc",
                                          bufs=4)
                    nc.sync.dma_start(ohtc[:], ohT_dram[:, c * CH:(c + 1) * CH, :])
                    ohT_ch[c] = ohtc

                dma_ohw_nat(0)
                dma_ohw_nat(1)
                dma_ift_oht(0)
                dma_ift_oht(1)
                dma_ohw_nat(2)
                dma_ohw_nat(3)
                dma_ift_oht(2)
                dma_ift_oht(3)

                def nat_pair(w, j):      # [P, 2, D] node-tile pair
                    g = w * T_W + 2 * j
                    return nat_ch[g // CH][:, (g % CH):(g % CH) + 2, :]

                def ohw_pair(w, j):      # [P, 2, P]
                    g = w * T_W + 2 * j
                    return ohw_ch[g // CH][:, (g % CH):(g % CH) + 2, :]

                def ifT_t(w, t):         # [P, 2, P]
                    g = w * T_W + t
                    return ifT_ch[g // CH][:, g % CH, :, :]

                def ohT_t(w, t):         # [P, P]
                    g = w * T_W + t
                    return ohT_ch[g // CH][:, g % CH, :]

                def ohw_t(w, t):         # [P, P]
                    g = w * T_W + t
                    return ohw_ch[g // CH][:, g % CH, :]

                # per-window state
                st = {}

                def emit_anchor_pair(w, j):
                    nc.tensor.matmul(st[w]["anchor_ps"][:], ohw_pair(w, j),
                                     nat_pair(w, j), start=(j == 0),
                                     stop=(j == NP - 1), perf_mode=PM.DoubleRow)

                def emit_window_setup(w):
                    # fv-pool bank (parity bufs=2): anchor accumulation, then
                    # fv, then wsum_lo — strictly sequential groups in [0:D).
                    # The etr bank holds the per-batch +e/-e columns
                    # [0:2*T_W) and the two bf16 transpose scratch regions.
                    anchor_ps = fv_ps_pool.tile(
                        [P, D], f32, name=f"anc{rep}_{w}", tag="fv_ps")
                    etr_ps = anchor_ps_pool.tile(
                        [P, 2 * T_W + P], f32, name=f"etr{rep}_{w}",
                        tag="anchor_ps")
                    out_sb = sbuf_pool.tile([P, 2 * D], f32, name=f"osb{rep}_{w}",
                                            tag="out_sb", bufs=2)
                    e_win = sbuf_pool.tile([P, T_W], f32, name=f"ew{rep}_{w}",
                                           tag="e_win", bufs=2)
                    z_win = sbuf_pool.tile([P, T_W], f32, name=f"zw{rep}_{w}",
                                           tag="z_win", bufs=2)
                    z8_win = sbuf_pool.tile([P, T_W], f32, name=f"z8{rep}_{w}",
                                            tag="z8_win", bufs=2)
                    zlo_win = sbuf_pool.tile([P, T_W], f32, name=f"zl{rep}_{w}",
                                             tag="zlo_win", bufs=2)
                    wsum_ps = wsum_ps_pool.tile([P, D + 1], f32,
                                                name=f"ws{rep}_{w}", tag="wsum_ps")
                    st[w] = dict(anchor_ps=anchor_ps, etr_ps=etr_ps,
                                 out_sb=out_sb,
                                 e_win=e_win, z_win=z_win, z8_win=z8_win,
                                 zlo_win=zlo_win, wsum_ps=wsum_ps)

                def emit_fv(w):
                    anchor_ps = st[w]["anchor_ps"]
                    etr_ps = st[w]["etr_ps"]
                    rc = rcnt_sb[:, w, :]
                    # anchor output half must be read before fv overwrites
                    nc.scalar.mul(st[w]["out_sb"][:, D:2 * D], anchor_ps[:],
                                  rc)
                    anchb = sbuf_pool.tile([P, D], bf16, name=f"a8{rep}_{w}",
                                           tag="anchb", bufs=1)
                    nc.vector.tensor_scalar(anchb[:], anchor_ps[:], rc,
                                            ANCH_SCALE, Alu.mult, Alu.mult)
                    anchT = sbuf_pool.tile([P, 2, P], fp8, name=f"aT{rep}_{w}",
                                           tag="anchT", bufs=1)
                    trbase = 2 * T_W
                    for db in range(2):
                        trp = (etr_ps[:, trbase + db * (P // 2):
                                      trbase + (db + 1) * (P // 2)]
                               .bitcast(bf16))
                        nc.tensor.transpose(trp, anchb[:, db * P:(db + 1) * P],
                                            idb_sb[:])
                    trall = etr_ps[:, trbase:trbase + P].bitcast(bf16)
                    nc.vector.tensor_copy(
                        anchT[:].rearrange("p k d -> p (k d)"), trall)
                    fv_ps = st[w]["anchor_ps"]
                    nc.tensor.matmul(fv_ps[:], anchT[:], wvT_sb[:],
                                     start=True, stop=True, perf_mode=PM.DoubleRow)
                    fv8 = sbuf_pool.tile([P, D], fp8, name=f"fv8{rep}_{w}",
                                         tag="fv8", bufs=1)
                    # fv8 = fv_ps * (FV_SCALE / (WU_SCALE*ANCH_SCALE)) + bv*FV_SCALE
                    nc.vector.scalar_tensor_tensor(
                        out=fv8[:], in0=fv_ps[:],
                        scalar=FV_SCALE / (WU_SCALE * ANCH_SCALE), in1=bvb_sb[:],
                        op0=Alu.mult, op1=Alu.add)
                    st[w]["fv8"] = fv8

                def emit_s_matmuls(w, b):
                    # transposed s: sT[do_half, node] per (half, q); the e
                    # reduction is then a ~free stationary-sT matmul with the
                    # we column as the 1-wide moving operand.
                    s_ps = s_ps_pool.tile([P, 2, SB, P], f32,
                                          name=f"sp{rep}_{w}_{b}", tag="s_ps")
                    fv8 = st[w]["fv8"]
                    for q in range(SB):
                        t = b * SB + q
                        oht_dup = ohT_t(w, t).unsqueeze(1).broadcast_to([P, 2, P])
                        for h in range(2):
                            nc.tensor.matmul(s_ps[:, h, q, :],
                                             wuT_sb[:, :, h * P:(h + 1) * P],
                                             ifT_t(w, t), start=True, stop=False,
                                             perf_mode=PM.DoubleRow)
                            fv_dup = (fv8[:, h * P:(h + 1) * P]
                                      .unsqueeze(1).broadcast_to([P, 2, P]))
                            nc.tensor.matmul(s_ps[:, h, q, :], fv_dup, oht_dup,
                                             start=False, stop=True,
                                             perf_mode=PM.DoubleRow)
                    s_sb = sbuf_pool.tile([P, 2, SB, P], bf16,
                                          name=f"ss{rep}_{w}_{b}", tag="s_sb",
                                          bufs=_ELAG + 1)
                    nc.scalar.activation(
                        s_sb[:].rearrange("p h q d -> p (h q d)"),
                        s_ps[:].rearrange("p h q d -> p (h q d)"),
                        Act.Sigmoid, scale=1.0 / WU_SCALE)
                    st[w][f"s_sb{b}"] = s_sb

                def emit_anchor_out(w):
                    # anchor output half; emitted mid-pipe so the window
                    # transition has less Act work
                    nc.scalar.mul(st[w]["out_sb"][:, D:2 * D],
                                  st[w]["anchor_ps"][:, 0:D], rcnt_sb[:, w, :])

                def emit_e_batch(w, b):
                    # writes +e and -e columns (negated we consts) so the z
                    # sigmoid pass needs a single activation per chunk
                    etr_ps = st[w]["etr_ps"]
                    s_sb = st[w].pop(f"s_sb{b}")
                    for q in range(SB):
                        for sgn in range(2):
                            ec = 8 * b + 4 * sgn + q
                            for h in range(2):
                                nc.tensor.matmul(etr_ps[:, ec:ec + 1],
                                                 s_sb[:, h, q, :],
                                                 web_sb[:, 2 * sgn + h:
                                                        2 * sgn + h + 1],
                                                 start=(h == 0), stop=(h == 1))
                    if _DEBUG:
                        nc.vector.tensor_copy(
                            st[w]["e_win"][:, b * SB:(b + 1) * SB],
                            etr_ps[:, 8 * b:8 * b + SB])

                def emit_z_chunk(w, b0, b1):
                    # one activation covers the interleaved [+e(4) -e(4)] cols
                    etr_ps, z_win = st[w]["etr_ps"], st[w]["z_win"]
                    nb = b1 - b0
                    spn = sbuf_pool.tile([P, nb, 2, SB], f32,
                                         name=f"zp{rep}_{w}_{b0}", tag="zch",
                                         bufs=4)
                    nc.scalar.activation(
                        spn[:].rearrange("p b s q -> p (b s q)"),
                        etr_ps[:, 8 * b0:8 * b1], Act.Sigmoid)
                    rn = sbuf_pool.tile([P, nb, SB], f32,
                                        name=f"zr{rep}_{w}_{b0}", tag="zrn",
                                        bufs=4)
                    nc.vector.reciprocal(rn[:], spn[:, :, 1, :])
                    nc.vector.tensor_tensor(
                        z_win[:, b0 * SB:b1 * SB].rearrange(
                            "p (b q) -> p b q", b=nb),
                        spn[:, :, 0, :], rn[:], Alu.mult)
                    c0, c1 = b0 * SB, b1 * SB
                    # two-term z: z8 = fp8-rounded z (kept in f32 so the scalar
                    # multiply with exact one-hots stores exactly in fp8),
                    # zlo = z - z8 (fp8-stored residual)
                    z8_win, zlo_win = st[w]["z8_win"], st[w]["zlo_win"]
                    z8q = sbuf_pool.tile([P, c1 - c0], fp8,
                                         name=f"zq{rep}_{w}_{c0}",
                                         tag="z8q", bufs=4)
                    nc.vector.tensor_copy(z8q[:], z_win[:, c0:c1])
                    nc.vector.tensor_copy(z8_win[:, c0:c1], z8q[:])
                    nc.vector.tensor_tensor(zlo_win[:, c0:c1], z_win[:, c0:c1],
                                            z8_win[:, c0:c1], Alu.subtract)

                def emit_wsum_pair(w, j):
                    wsum_ps = st[w]["wsum_ps"]
                    z8_win, zlo_win = st[w]["z8_win"], st[w]["zlo_win"]
                    wlo_ps = st[w]["anchor_ps"]  # same bank: anchor->fv->wlo
                    ohz = sbuf_pool.tile([P, 2, P], fp8, name=f"oz{rep}_{w}_{j}",
                                         tag="ohz", bufs=4)
                    ohzlo = sbuf_pool.tile([P, 2, P], fp8, name=f"ol{rep}_{w}_{j}",
                                           tag="ohzlo", bufs=4)
                    for k in range(2):
                        t = 2 * j + k
                        nc.vector.tensor_scalar(ohz[:, k, :], ohw_t(w, t),
                                                z8_win[:, t:t + 1], None, Alu.mult)
                        nc.vector.tensor_scalar(ohzlo[:, k, :], ohw_t(w, t),
                                                zlo_win[:, t:t + 1], None,
                                                Alu.mult)
                    nc.tensor.matmul(wsum_ps[:, 0:D], ohz[:], nat_pair(w, j),
                                     start=(j == 0), stop=(j == NP - 1),
                                     perf_mode=PM.DoubleRow)
                    # NOTE: further accumulation groups in the same PSUM bank must
                    # ride the first group's start/stop envelope (start=False;
                    # the j==0 start above zeroes the whole bank) — interleaved
                    # groups with their own start wipe the bank's other region.
                    nc.tensor.matmul(wsum_ps[:, D:D + 1], ohz[:],
                                     one_sb[:].unsqueeze(2),
                                     start=False, stop=False,
                                     perf_mode=PM.DoubleRow,
                                     skip_group_check=True)
                    nc.tensor.matmul(wsum_ps[:, D:D + 1], ohzlo[:],
                                     one_sb[:].unsqueeze(2),
                                     start=False, stop=(j == NP - 1),
                                     perf_mode=PM.DoubleRow,
                                     skip_group_check=True)
                    nc.tensor.matmul(wlo_ps[:], ohzlo[:], nat_pair(w, j),
                                     start=(j == 0), stop=(j == NP - 1),
                                     perf_mode=PM.DoubleRow)

                def emit_output(w):
                    wsum_ps, out_sb = st[w]["wsum_ps"], st[w]["out_sb"]
                    wlo_ps = st[w]["anchor_ps"]
                    den = sbuf_pool.tile([P, 1], f32, name=f"dn{rep}_{w}",
                                         tag="den", bufs=2)
                    nc.vector.tensor_scalar(den[:], wsum_ps[:, D:D + 1], 1e-30,
                                            None, Alu.max)
                    rden = sbuf_pool.tile([P, 1], f32, name=f"rd{rep}_{w}",
                                          tag="rden", bufs=2)
                    nc.vector.reciprocal(rden[:], den[:])
                    wlo_sc = sbuf_pool.tile([P, D], f32, name=f"wl{rep}_{w}",
                                            tag="wlo_sc", bufs=2)
                    nc.scalar.mul(wlo_sc[:], wlo_ps[:], rden[:])
                    nc.vector.scalar_tensor_tensor(
                        out=out_sb[:, 0:D], in0=wsum_ps[:, 0:D], scalar=rden[:],
                        in1=wlo_sc[:], op0=Alu.mult, op1=Alu.add)
                    nc.sync.dma_start(out_dram[w], out_sb[:])
                    if _DEBUG:
                        dbg = sbuf_pool.tile([P, 2 * T_W], f32,
                                             name=f"dbg{rep}_{w}", tag="dbg",
                                             bufs=2)
                        nc.vector.tensor_copy(dbg[:, 0:T_W], st[w]["e_win"][:])
                        nc.vector.tensor_copy(dbg[:, T_W:], st[w]["z_win"][:])
                        nc.sync.dma_start(dbg_dram[w], dbg[:])

                # ---------------- emission schedule ----------------
                # Software-pipelined: e-matmuls lag their sigmoid batch by
                # E_LAG so the in-order PE stream never stalls on Act; z runs
                # in fine-grained chunks (ZB batches) so wsum pairs drain
                # incrementally; the small leftover tail of window 0 overlaps
                # window 1's anchor pass (anchor banks alternate by parity).
                E_LAG = _ELAG
                ZB = _ZB
                bd = list(range(0, NB, ZB)) + [NB]
                if bd[-2] == NB:
                    bd.pop()

                def pass2b(w):
                    from collections import deque
                    seq, ready = [], deque()

                    def drain(n):
                        for _ in range(n):
                            if ready:
                                seq.append(ready.popleft())

                    def after_e(eb):
                        seq.append(("e", w, eb))
                        drain(_DRAIN)
                        if (eb + 1) in bd[1:]:
                            c = bd.index(eb + 1) - 1
                            seq.append(("z", w, c))
                            for j in range(bd[c] * 2, bd[c + 1] * 2):
                                ready.append(("w", w, j))

                    for b in range(NB):
                        seq.append(("s", w, b))
                        drain(_DRAIN)
                        if b - E_LAG >= 0:
                            after_e(b - E_LAG)
                    for eb in range(max(0, NB - E_LAG), NB):
                        after_e(eb)
                    return seq, list(ready)

                def dispatch(seq):
                    for kind, w, i in seq:
                        if kind == "s":
                            emit_s_matmuls(w, i)
                        elif kind == "e":
                            emit_e_batch(w, i)
                        elif kind == "z":
                            emit_z_chunk(w, bd[i], bd[i + 1])
                        elif kind == "w":
                            emit_wsum_pair(w, i)
                        elif kind == "a":
                            emit_anchor_pair(w, i)

                emit_window_setup(0)
                emit_window_setup(1)
                for j in range(NP):
                    emit_anchor_pair(0, j)
                emit_fv(0)
                seq0, tail0 = pass2b(0)
                q1, q2 = len(seq0) // 4, len(seq0) // 2
                dispatch(seq0[:q1])
                dispatch(_weave2(seq0[q1:q2],
                                 [("a", 1, j) for j in range(NP)]))
                emit_fv(1)
                dispatch(seq0[q2:])
                dispatch(tail0)
                emit_output(0)
                seq1, tail1 = pass2b(1)
                dispatch(seq1)
                dispatch(tail1)
                emit_output(1)

            if loop_repeat is not None:
                import inspect as _insp
                _kw = {}
                if "staggered_reset" in _insp.signature(tc.For_i).parameters:
                    _kw["staggered_reset"] = _STAGGER
                with tc.For_i(0, loop_repeat, 1, **_kw):
                    for u in range(_UNROLL):
                        body(f"L{u}")
            else:
                for rep in range(repeat):
                    body(rep)

    return nc


def _weave2(a_items, b_items):
    """Proportionally interleave two lists, preserving each list's order."""
    na, nb = len(a_items), len(b_items)
    out = []
    ai = bi = 0
    while ai < na or bi < nb:
        if ai < na and (bi >= nb or ai * nb <= bi * na):
            out.append(a_items[ai]); ai += 1
        else:
            out.append(b_items[bi]); bi += 1
    return out


def _prepare(ifeat, Wu, Wv, bv, we, seg_ids):
    """Host-side shard + pad + layout. Returns (T_W, in_maps)."""
    ifeat = np.asarray(ifeat, dtype=np.float32)
    Wu = np.asarray(Wu, dtype=np.float32)
    Wv = np.asarray(Wv, dtype=np.float32)
    bv = np.asarray(bv, dtype=np.float32)
    we = np.asarray(we, dtype=np.float32)
    seg_ids = np.asarray(seg_ids)

    W = N_WINDOWS
    bounds = np.searchsorted(
        seg_ids, np.arange(0, B + 1, SEGS_PER_WINDOW), side="left")
    n_w = np.diff(bounds)
    T_W = max(4, int(-(-int(n_w.max()) // P)))
    T_W = ((T_W + 3) // 4) * 4
    NT = W_PER_CORE * T_W

    win = (seg_ids // SEGS_PER_WINDOW).astype(np.int64)
    pos = np.arange(N, dtype=np.int64) - bounds[win]
    sloc = (seg_ids % SEGS_PER_WINDOW).astype(np.int64)

    if8 = ifeat.astype(F8)
    # error-diffused fp8 rounding of nat: within each segment (nodes sorted),
    # carry the rounding residual forward so segment sums are nearly exact.
    natq = np.empty((N, D), dtype=F8)
    carry = np.zeros(D, dtype=np.float32)
    seg_np = np.asarray(seg_ids, dtype=np.int64)
    prev = -1
    for i in range(N):
        s = seg_np[i]
        if s != prev:
            carry[:] = 0.0
            prev = s
        v = ifeat[i] + carry
        q = v.astype(F8)
        carry = v - q.astype(np.float32)
        natq[i] = q
    natA = np.zeros((W, T_W * P, D), dtype=F8)
    natA[win, pos, :] = natq
    ifA = np.zeros((W, T_W * P, D), dtype=F8)   # nearest-rounded for fc_u
    ifA[win, pos, :] = if8
    ohwA = np.zeros((W, T_W * P, P), dtype=F8)
    ohwA[win, pos, sloc] = 1.0

    counts = np.bincount(np.asarray(seg_ids, dtype=np.int64), minlength=B)
    rcnt = (1.0 / np.maximum(counts, 1)).astype(np.float32).reshape(W, P, 1)

    wuT8 = (np.ascontiguousarray(Wu.T) * WU_SCALE).reshape(2, P, D)
    wuT8 = np.ascontiguousarray(wuT8.transpose(1, 0, 2)).astype(F8)  # [P,2,D]
    wvT8 = (np.ascontiguousarray(Wv.T) * WU_SCALE).reshape(2, P, D)
    wvT8 = np.ascontiguousarray(wvT8.transpose(1, 0, 2)).astype(F8)
    web = np.concatenate([we.reshape(2, P).T, -we.reshape(2, P).T],
                         axis=1).astype(BF)  # [dlo, (+h0,+h1,-h0,-h1)]
    bvb8 = np.tile(bv * FV_SCALE, (P, 1)).astype(np.float32)
    idb = np.eye(P, dtype=BF)
    one8 = np.ones((P, 2), dtype=F8)

    in_maps = []
    for c in range(N_CORES):
        X = natA[2 * c:2 * c + 2].reshape(W_PER_CORE, T_W, P, D)
        # nat [lane, (w,t), d]
        natp = np.ascontiguousarray(
            X.transpose(2, 0, 1, 3).reshape(P, NT, D))
        # ifT [d_lo, (w,t), kb, lane]
        Y = ifA[2 * c:2 * c + 2].reshape(W_PER_CORE, T_W, P, 2, P)
        iftp = np.ascontiguousarray(
            Y.transpose(4, 0, 1, 3, 2).reshape(P, NT, 2, P))
        O = ohwA[2 * c:2 * c + 2].reshape(W_PER_CORE, T_W, P, P)
        ohwp = np.ascontiguousarray(
            O.transpose(2, 0, 1, 3).reshape(P, NT, P))
        ohtp = np.ascontiguousarray(
            O.transpose(3, 0, 1, 2).reshape(P, NT, P))
        in_maps.append({
            "natp": natp, "iftp": iftp, "ohwp": ohwp, "ohtp": ohtp,
            "wuT8": wuT8, "wvT8": wvT8, "web": web, "bvb8": bvb8,
            "idb": idb, "one8": one8,
            "rcnt": rcnt[2 * c:2 * c + 2],
        })
    return T_W, in_maps


_DEBUG = False
_LAST = {}


def _run(ifeat, Wu, Wv, bv, we, seg_ids, trace=False):
    from concourse.bass_utils import run_bass_kernel_spmd

    T_W, in_maps = _prepare(ifeat, Wu, Wv, bv, we, seg_ids)
    nc = _build(T_W)
    _split_sync_waits(nc)
    res = run_bass_kernel_spmd(nc, in_maps, list(range(N_CORES)), trace=trace)
    _LAST["res"] = res
    _LAST["T_W"] = T_W
    _LAST["nc"] = nc
    _LAST["in_maps"] = in_maps

    out = np.empty((B, 2 * D), dtype=np.float32)
    for c in range(N_CORES):
        core_out = res.results[c]["out"]  # [W_PER_CORE, P, 2D]
        for wl in range(W_PER_CORE):
            w = c * W_PER_CORE + wl
            out[w * SEGS_PER_WINDOW:(w + 1) * SEGS_PER_WINDOW, :] = core_out[wl]
    return out


def kernel(ifeat, Wu, Wv, bv, we, seg_ids):
    return _run(ifeat, Wu, Wv, bv, we, seg_ids, trace=False)


# revision 51
# speedup vs baseline: 2.9634x; 1.0754x over previous
"""Trainium2 Bass kernel for nn_AttnReadout (segment attention readout).

Computation (reference):
    anchor[b]  = mean of ifeat rows in segment b                  [B, D]
    e[i]       = sigmoid(ifeat @ Wu.T + (anchor @ Wv.T + bv)[seg]) @ we
    alpha      = segment_softmax(e)
    rst[b]     = sum_i alpha[i] * ifeat[i]                        [B, D]
    out        = concat([rst, anchor], axis=1)                    [B, 2D]

Sharding: 2048 segments -> 8 cores x 2 windows of 128 contiguous segments.
Nodes (sorted by segment) are padded per-window to T_W tiles of 128 rows.

All heavy matmuls run in fp8e4 with MatmulPerfMode.DoubleRow (2 k-tiles per
instruction, 0.5 cycles per output column):
  - anchor / weighted-sum segment reductions pair ADJACENT node tiles as the
    two k-tiles (one-hot pair as stationary, feature pair as moving).
  - fc_u contracts the two 128-feature halves as k-tiles.
  - the fv gather duplicates its single 128-seg k-tile via stride-0 APs and
    pre-halves fv to compensate.
One-hot matrices in both layouts (ohw [node,seg] and ohT [seg,node]) are
host-prepared from seg_ids and DMAed as fp8 (no on-device transposes).
Sigmoid runs on batches of 4 node tiles from a 2-bank PSUM region to
amortize the activation-engine access latency; z = exp(e) uses the
sigmoid(e)/sigmoid(-e) identity to stay on one ACT table.

Scaling: Wu.T and Wv.T are stored x16 (good fp8 range), anchor means x4,
fv stored as 8*(fv+bv) (halved for the stride-0 duplication), and the
sigmoid applies scale=1/16 to undo it. The z scale cancels in rst/denom.
"""

import numpy as np
import ml_dtypes

N = 102400
D = 256
B = 2048
N_CORES = 8
W_PER_CORE = 2
N_WINDOWS = N_CORES * W_PER_CORE  # 16
SEGS_PER_WINDOW = B // N_WINDOWS  # 128
P = 128
BF = ml_dtypes.bfloat16
F8 = ml_dtypes.float8_e4m3fn

WU_SCALE = 16.0
ANCH_SCALE = 4.0
FV_SCALE = 8.0  # fv stored as FV_SCALE*(fv+bv); doubled by dup -> 16
SB = 4          # sigmoid batch (node tiles per activation)


def _apply_tile_patch():
    """Split TileContext's multi-wait tail drain into single-wait drains
    (this walrus build rejects >1 sync wait on a Drain instruction)."""
    import concourse.tile as tile_mod
    from concourse.vector_clock import ScopedClock

    if getattr(tile_mod.TileContext, "_drain_wait_split_patch", False):
        return

    def _patched(self, tick_clock, wait_clock):
        nc = self.nc
        drain_inst = nc.sync.drain()
        wait_clock.add_sem_waits(
            drain_inst.ins, ScopedClock({None: tick_clock.global_clock})
        )
        si = drain_inst.ins.sync_info
        waits = list(si.on_wait) if si is not None else []
        if len(waits) > 1:
            SyncInfo = type(si)
            drain_inst.ins.sync_info = SyncInfo(
                on_wait=[waits[0]], on_update=list(si.on_update)
            )
            for w in waits[1:]:
                extra = nc.sync.drain()
                extra.ins.sync_info = SyncInfo(on_wait=[w], on_update=[])

        nc.all_engine_barrier()
        assert self.sems is not None
        popped = nc._tile_sem_poison_stack.pop()
        assert popped is self._sem_poison
        nc.clear_and_free_semaphores(list(self.sems.allocated().values()))
        nc.all_engine_barrier()

    tile_mod.TileContext._drain_and_barrier = _patched
    tile_mod.TileContext._drain_wait_split_patch = True


def _split_sync_waits(nc, limit=1):
    """Split >limit sync waits per instruction into preceding single-wait
    EventSemaphore carriers on the same engine (walrus build limit)."""
    import concourse.mybir as mybir

    n_new = 0
    for _, bassbb in nc.bb_map.items():
        insts = bassbb.bb.instructions  # live list
        snapshot = list(insts)
        offset = 0
        for pos, inst in enumerate(snapshot):
            si = getattr(inst, "sync_info", None)
            if si is None:
                continue
            waits = list(si.on_wait)
            if len(waits) <= limit:
                continue
            SyncInfo = type(si)
            inst.sync_info = SyncInfo(
                on_wait=waits[:limit], on_update=list(si.on_update))
            carriers = []
            for w in waits[limit:]:
                c = mybir.InstEventSemaphore(
                    name=f"WSPLIT-{nc.next_id()}", ins=[], outs=[])
                c.engine = inst.engine
                c.sync_info = SyncInfo(on_wait=[w], on_update=[])
                carriers.append(c)
            insts[pos + offset:pos + offset] = carriers
            offset += len(carriers)
            n_new += len(carriers)
    return n_new


_PACK = {}
_STAGGER = False
_UNROLL = 1
_ELAG = 2
_ZB = 4
_DRAIN = 4
_Q1 = 2
_Q2 = 4


def _build(T_W, repeat=1, loop_repeat=None):
    """Build the single-core SPMD Bass program; T_W must be a multiple of 4.

    Uses _PACK["bases"] (canonical per-tile 32-aligned segment-block base,
    identical across cores) and _PACK["width"] set by _prepare: the one-hot
    of tile t only has nonzero columns in [base(t), base(t)+width)."""
    import contextlib
    import concourse.bass as bass
    import concourse.mybir as mybir
    from concourse.tile import TileContext

    _apply_tile_patch()

    f32 = mybir.dt.float32
    bf16 = mybir.dt.bfloat16
    fp8 = mybir.dt.float8e4
    Alu = mybir.AluOpType
    Act = mybir.ActivationFunctionType
    PM = mybir.MatmulPerfMode

    assert T_W % 4 == 0
    CH = T_W // 2            # tiles per DMA chunk (2 chunks per window), even
    NT = W_PER_CORE * T_W
    NB = T_W // SB           # sigmoid batches per window
    NB0 = NB // 2            # batches in z-chunk 0
    TC0 = NB0 * SB           # tiles in z-chunk 0
    NP = T_W // 2            # node-tile pairs per window
    NP0 = TC0 // 2           # pairs fully covered by z-chunk 0

    nc = bass.Bass("TRN2", num_devices=N_CORES)

    nat_dram = nc.dram_tensor("natp", [P, NT, D], fp8, kind="ExternalInput")
    ifT_dram = nc.dram_tensor("iftp", [P, NT, 2, P], fp8, kind="ExternalInput")
    ohw_dram = nc.dram_tensor("ohwp", [P, NT, P], fp8, kind="ExternalInput")
    ohT_dram = nc.dram_tensor("ohtp", [P, NT, P], fp8, kind="ExternalInput")
    wuT_dram = nc.dram_tensor("wuT8", [P, 2, D], fp8, kind="ExternalInput")
    wvT_dram = nc.dram_tensor("wvT8", [P, 2, D], fp8, kind="ExternalInput")
    web_dram = nc.dram_tensor("web", [P, 4], bf16, kind="ExternalInput")
    bvb_dram = nc.dram_tensor("bvb8", [P, D], f32, kind="ExternalInput")
    idb_dram = nc.dram_tensor("idb", [P, P], bf16, kind="ExternalInput")
    one_dram = nc.dram_tensor("one8", [P, 2], fp8, kind="ExternalInput")
    rcnt_dram = nc.dram_tensor("rcnt", [W_PER_CORE, P, 1], f32,
                               kind="ExternalInput")
    out_dram = nc.dram_tensor("out", [W_PER_CORE, P, 2 * D], f32,
                              kind="ExternalOutput")
    dbg_dram = None
    if _DEBUG:
        dbg_dram = nc.dram_tensor("dbg", [W_PER_CORE, P, 2 * T_W], f32,
                                  kind="ExternalOutput")

    with TileContext(nc) as tc:
        with contextlib.ExitStack() as ctx:
            const_pool = ctx.enter_context(tc.tile_pool(name="const", bufs=1))
            data_pool = ctx.enter_context(tc.tile_pool(name="data", bufs=1))
            sbuf_pool = ctx.enter_context(tc.tile_pool(name="sbuf", bufs=1))
            # PSUM: s_ps 2x2 banks + anchor 1 (tile also hosts the e columns
            # and transpose scratch) + wsum 1 + fv/wlo 2 (cross-rep overlap:
            # next rep's fv must not wait on this rep's final output) = 8.
            anchor_ps_pool = ctx.enter_context(
                tc.tile_pool(name="anchor_ps", bufs=1, space="PSUM"))
            wsum_ps_pool = ctx.enter_context(
                tc.tile_pool(name="wsum_ps", bufs=1, space="PSUM"))
            s_ps_pool = ctx.enter_context(
                tc.tile_pool(name="s_ps", bufs=2, space="PSUM"))
            fv_ps_pool = ctx.enter_context(
                tc.tile_pool(name="fv_ps", bufs=2, space="PSUM"))

            # ---- constants ----
            wuT_sb = const_pool.tile([P, 2, D], fp8, name="wuT_sb", tag="wuT_sb")
            nc.sync.dma_start(wuT_sb[:], wuT_dram[:])
            wvT_sb = const_pool.tile([P, 2, D], fp8, name="wvT_sb", tag="wvT_sb")
            nc.sync.dma_start(wvT_sb[:], wvT_dram[:])
            web_sb = const_pool.tile([P, 4], bf16, name="web_sb", tag="web_sb")
            nc.sync.dma_start(web_sb[:], web_dram[:])
            bvb_sb = const_pool.tile([P, D], f32, name="bvb_sb", tag="bvb_sb")
            nc.sync.dma_start(bvb_sb[:], bvb_dram[:])
            idb_sb = const_pool.tile([P, P], bf16, name="idb_sb", tag="idb_sb")
            nc.sync.dma_start(idb_sb[:], idb_dram[:])
            one_sb = const_pool.tile([P, 2], fp8, name="one_sb", tag="one_sb")
            nc.sync.dma_start(one_sb[:], one_dram[:])
            rcnt_sb = const_pool.tile([P, W_PER_CORE, 1], f32, name="rcnt_sb",
                                      tag="rcnt_sb")
            nc.sync.dma_start(
                rcnt_sb[:], rcnt_dram[:].rearrange("w p one -> p w one"))

            def body(rep):
                # window data, 2 chunks per window, all resident.
                # DMA emission follows consumption order: the anchor pass of a
                # window needs ohw+nat of both its chunks first; ifT/ohT feed
                # the later s-pipeline.
                nat_ch, ifT_ch, ohw_ch, ohT_ch = {}, {}, {}, {}

                def dma_ohw_nat(c):
                    ohwc = data_pool.tile([P, CH, P], fp8,
                                          name=f"ohwc{rep}_{c}", tag="ohwc",
                                          bufs=4)
                    nc.sync.dma_start(ohwc[:], ohw_dram[:, c * CH:(c + 1) * CH, :])
                    ohw_ch[c] = ohwc
                    natc = data_pool.tile([P, CH, D], fp8,
                                          name=f"natc{rep}_{c}", tag="natc",
                                          bufs=4)
                    nc.sync.dma_start(natc[:], nat_dram[:, c * CH:(c + 1) * CH, :])
                    nat_ch[c] = natc

                def dma_ift_oht(c):
                    iftc = data_pool.tile([P, CH, 2, P], fp8,
                                          name=f"iftc{rep}_{c}", tag="iftc",
                                          bufs=4)
                    nc.sync.dma_start(iftc[:], ifT_dram[:, c * CH:(c + 1) * CH, :, :])
                    ifT_ch[c] = iftc
                    ohtc = data_pool.tile([P, CH, P], fp8,
                                          name=f"ohtc{rep}_{c}", tag="ohtc",
                                          bufs=4)
                    nc.sync.dma_start(ohtc[:], ohT_dram[:, c * CH:(c + 1) * CH, :])
                    ohT_ch[c] = ohtc

                dma_ohw_nat(0)
                dma_ohw_nat(1)
                dma_ift_oht(0)
                dma_ift_oht(1)
                dma_ohw_nat(2)
                dma_ohw_nat(3)
                dma_ift_oht(2)
                dma_ift_oht(3)

                def nat_pair(w, j):      # [P, 2, D] node-tile pair
                    g = w * T_W + 2 * j
                    return nat_ch[g // CH][:, (g % CH):(g % CH) + 2, :]

                def ohw_pair(w, j):      # [P, 2, P]
                    g = w * T_W + 2 * j
                    return ohw_ch[g // CH][:, (g % CH):(g % CH) + 2, :]

                def ifT_t(w, t):         # [P, 2, P]
                    g = w * T_W + t
                    return ifT_ch[g // CH][:, g % CH, :, :]

                def ohT_t(w, t):         # [P, P]
                    g = w * T_W + t
                    return ohT_ch[g // CH][:, g % CH, :]

                def ohw_t(w, t):         # [P, P]
                    g = w * T_W + t
                    return ohw_ch[g // CH][:, g % CH, :]

                # per-window state
                st = {}

                def emit_anchor_pair(w, j):
                    nc.tensor.matmul(st[w]["anchor_ps"][:], ohw_pair(w, j),
                                     nat_pair(w, j), start=(j == 0),
                                     stop=(j == NP - 1), perf_mode=PM.DoubleRow)

                def emit_window_setup(w):
                    # fv-pool bank (parity bufs=2): anchor accumulation, then
                    # fv, then wsum_lo — strictly sequential groups in [0:D).
                    # The etr bank holds the per-batch +e/-e columns
                    # [0:2*T_W) and the two bf16 transpose scratch regions.
                    anchor_ps = fv_ps_pool.tile(
                        [P, D], f32, name=f"anc{rep}_{w}", tag="fv_ps")
                    etr_ps = anchor_ps_pool.tile(
                        [P, 2 * T_W + P], f32, name=f"etr{rep}_{w}",
                        tag="anchor_ps")
                    out_sb = sbuf_pool.tile([P, 2 * D], f32, name=f"osb{rep}_{w}",
                                            tag="out_sb", bufs=2)
                    e_win = sbuf_pool.tile([P, T_W], f32, name=f"ew{rep}_{w}",
                                           tag="e_win", bufs=2)
                    z_win = sbuf_pool.tile([P, T_W], f32, name=f"zw{rep}_{w}",
                                           tag="z_win", bufs=2)
                    z8_win = sbuf_pool.tile([P, T_W], f32, name=f"z8{rep}_{w}",
                                            tag="z8_win", bufs=2)
                    zlo_win = sbuf_pool.tile([P, T_W], f32, name=f"zl{rep}_{w}",
                                             tag="zlo_win", bufs=2)
                    wsum_ps = wsum_ps_pool.tile([P, D + 1], f32,
                                                name=f"ws{rep}_{w}", tag="wsum_ps")
                    st[w] = dict(anchor_ps=anchor_ps, etr_ps=etr_ps,
                                 out_sb=out_sb,
                                 e_win=e_win, z_win=z_win, z8_win=z8_win,
                                 zlo_win=zlo_win, wsum_ps=wsum_ps)

                def emit_fv(w):
                    anchor_ps = st[w]["anchor_ps"]
                    etr_ps = st[w]["etr_ps"]
                    rc = rcnt_sb[:, w, :]
                    # anchor output half must be read before fv overwrites
                    nc.scalar.mul(st[w]["out_sb"][:, D:2 * D], anchor_ps[:],
                                  rc)
                    anchb = sbuf_pool.tile([P, D], bf16, name=f"a8{rep}_{w}",
                                           tag="anchb", bufs=1)
                    nc.vector.tensor_scalar(anchb[:], anchor_ps[:], rc,
                                            ANCH_SCALE, Alu.mult, Alu.mult)
                    anchT = sbuf_pool.tile([P, 2, P], fp8, name=f"aT{rep}_{w}",
                                           tag="anchT", bufs=1)
                    trbase = 2 * T_W
                    for db in range(2):
                        trp = (etr_ps[:, trbase + db * (P // 2):
                                      trbase + (db + 1) * (P // 2)]
                               .bitcast(bf16))
                        nc.tensor.transpose(trp, anchb[:, db * P:(db + 1) * P],
                                            idb_sb[:])
                    trall = etr_ps[:, trbase:trbase + P].bitcast(bf16)
                    nc.vector.tensor_copy(
                        anchT[:].rearrange("p k d -> p (k d)"), trall)
                    fv_ps = st[w]["anchor_ps"]
                    nc.tensor.matmul(fv_ps[:], anchT[:], wvT_sb[:],
                                     start=True, stop=True, perf_mode=PM.DoubleRow)
                    fv8 = sbuf_pool.tile([P, D], fp8, name=f"fv8{rep}_{w}",
                                         tag="fv8", bufs=1)
                    # fv8 = fv_ps * (FV_SCALE / (WU_SCALE*ANCH_SCALE)) + bv*FV_SCALE
                    nc.vector.scalar_tensor_tensor(
                        out=fv8[:], in0=fv_ps[:],
                        scalar=FV_SCALE / (WU_SCALE * ANCH_SCALE), in1=bvb_sb[:],
                        op0=Alu.mult, op1=Alu.add)
                    st[w]["fv8"] = fv8

                def emit_s_matmuls(w, b):
                    # transposed s: sT[do_half, node] per (half, q); the e
                    # reduction is then a ~free stationary-sT matmul with the
                    # we column as the 1-wide moving operand.
                    s_ps = s_ps_pool.tile([P, 2, SB, P], f32,
                                          name=f"sp{rep}_{w}_{b}", tag="s_ps")
                    fv8 = st[w]["fv8"]
                    for q in range(SB):
                        t = b * SB + q
                        oht_dup = ohT_t(w, t).unsqueeze(1).broadcast_to([P, 2, P])
                        for h in range(2):
                            nc.tensor.matmul(s_ps[:, h, q, :],
                                             wuT_sb[:, :, h * P:(h + 1) * P],
                                             ifT_t(w, t), start=True, stop=False,
                                             perf_mode=PM.DoubleRow)
                            fv_dup = (fv8[:, h * P:(h + 1) * P]
                                      .unsqueeze(1).broadcast_to([P, 2, P]))
                            nc.tensor.matmul(s_ps[:, h, q, :], fv_dup, oht_dup,
                                             start=False, stop=True,
                                             perf_mode=PM.DoubleRow)
                    s_sb = sbuf_pool.tile([P, 2, SB, P], bf16,
                                          name=f"ss{rep}_{w}_{b}", tag="s_sb",
                                          bufs=_ELAG + 1)
                    nc.scalar.activation(
                        s_sb[:].rearrange("p h q d -> p (h q d)"),
                        s_ps[:].rearrange("p h q d -> p (h q d)"),
                        Act.Sigmoid, scale=1.0 / WU_SCALE)
                    st[w][f"s_sb{b}"] = s_sb

                def emit_anchor_out(w):
                    # anchor output half; emitted mid-pipe so the window
                    # transition has less Act work
                    nc.scalar.mul(st[w]["out_sb"][:, D:2 * D],
                                  st[w]["anchor_ps"][:, 0:D], rcnt_sb[:, w, :])

                def emit_e_batch(w, b):
                    # writes +e and -e columns (negated we consts) so the z
                    # sigmoid pass needs a single activation per chunk
                    etr_ps = st[w]["etr_ps"]
                    s_sb = st[w].pop(f"s_sb{b}")
                    for q in range(SB):
                        for sgn in range(2):
                            ec = 8 * b + 4 * sgn + q
                            for h in range(2):
                                nc.tensor.matmul(etr_ps[:, ec:ec + 1],
                                                 s_sb[:, h, q, :],
                                                 web_sb[:, 2 * sgn + h:
                                                        2 * sgn + h + 1],
                                                 start=(h == 0), stop=(h == 1))
                    if _DEBUG:
                        nc.vector.tensor_copy(
                            st[w]["e_win"][:, b * SB:(b + 1) * SB],
                            etr_ps[:, 8 * b:8 * b + SB])

                def emit_z_chunk(w, b0, b1):
                    # one activation covers the interleaved [+e(4) -e(4)] cols
                    etr_ps, z_win = st[w]["etr_ps"], st[w]["z_win"]
                    nb = b1 - b0
                    spn = sbuf_pool.tile([P, nb, 2, SB], f32,
                                         name=f"zp{rep}_{w}_{b0}", tag="zch",
                                         bufs=4)
                    nc.scalar.activation(
                        spn[:].rearrange("p b s q -> p (b s q)"),
                        etr_ps[:, 8 * b0:8 * b1], Act.Sigmoid)
                    rn = sbuf_pool.tile([P, nb, SB], f32,
                                        name=f"zr{rep}_{w}_{b0}", tag="zrn",
                                        bufs=4)
                    nc.vector.reciprocal(rn[:], spn[:, :, 1, :])
                    nc.vector.tensor_tensor(
                        z_win[:, b0 * SB:b1 * SB].rearrange(
                            "p (b q) -> p b q", b=nb),
                        spn[:, :, 0, :], rn[:], Alu.mult)
                    c0, c1 = b0 * SB, b1 * SB
                    # two-term z: z8 = fp8-rounded z (kept in f32 so the scalar
                    # multiply with exact one-hots stores exactly in fp8),
                    # zlo = z - z8 (fp8-stored residual)
                    z8_win, zlo_win = st[w]["z8_win"], st[w]["zlo_win"]
                    z8q = sbuf_pool.tile([P, c1 - c0], fp8,
                                         name=f"zq{rep}_{w}_{c0}",
                                         tag="z8q", bufs=4)
                    nc.vector.tensor_copy(z8q[:], z_win[:, c0:c1])
                    nc.vector.tensor_copy(z8_win[:, c0:c1], z8q[:])
                    nc.vector.tensor_tensor(zlo_win[:, c0:c1], z_win[:, c0:c1],
                                            z8_win[:, c0:c1], Alu.subtract)

                def emit_wsum_pair(w, j):
                    wsum_ps = st[w]["wsum_ps"]
                    z8_win, zlo_win = st[w]["z8_win"], st[w]["zlo_win"]
                    wlo_ps = st[w]["anchor_ps"]  # same bank: anchor->fv->wlo
                    ohz = sbuf_pool.tile([P, 2, P], fp8, name=f"oz{rep}_{w}_{j}",
                                         tag="ohz", bufs=4)
                    ohzlo = sbuf_pool.tile([P, 2, P], fp8, name=f"ol{rep}_{w}_{j}",
                                           tag="ohzlo", bufs=4)
                    for k in range(2):
                        t = 2 * j + k
                        nc.vector.tensor_scalar(ohz[:, k, :], ohw_t(w, t),
                                                z8_win[:, t:t + 1], None, Alu.mult)
                        nc.vector.tensor_scalar(ohzlo[:, k, :], ohw_t(w, t),
                                                zlo_win[:, t:t + 1], None,
                                                Alu.mult)
                    nc.tensor.matmul(wsum_ps[:, 0:D], ohz[:], nat_pair(w, j),
                                     start=(j == 0), stop=(j == NP - 1),
                                     perf_mode=PM.DoubleRow)
                    # NOTE: further accumulation groups in the same PSUM bank must
                    # ride the first group's start/stop envelope (start=False;
                    # the j==0 start above zeroes the whole bank) — interleaved
                    # groups with their own start wipe the bank's other region.
                    nc.tensor.matmul(wsum_ps[:, D:D + 1], ohz[:],
                                     one_sb[:].unsqueeze(2),
                                     start=False, stop=False,
                                     perf_mode=PM.DoubleRow,
                                     skip_group_check=True)
                    nc.tensor.matmul(wsum_ps[:, D:D + 1], ohzlo[:],
                                     one_sb[:].unsqueeze(2),
                                     start=False, stop=(j == NP - 1),
                                     perf_mode=PM.DoubleRow,
                                     skip_group_check=True)
                    nc.tensor.matmul(wlo_ps[:], ohzlo[:], nat_pair(w, j),
                                     start=(j == 0), stop=(j == NP - 1),
                                     perf_mode=PM.DoubleRow)

                def emit_output(w):
                    wsum_ps, out_sb = st[w]["wsum_ps"], st[w]["out_sb"]
                    wlo_ps = st[w]["anchor_ps"]
                    den = sbuf_pool.tile([P, 1], f32, name=f"dn{rep}_{w}",
                                         tag="den", bufs=2)
                    nc.vector.tensor_scalar(den[:], wsum_ps[:, D:D + 1], 1e-30,
                                            None, Alu.max)
                    rden = sbuf_pool.tile([P, 1], f32, name=f"rd{rep}_{w}",
                                          tag="rden", bufs=2)
                    nc.vector.reciprocal(rden[:], den[:])
                    wlo_sc = sbuf_pool.tile([P, D], f32, name=f"wl{rep}_{w}",
                                            tag="wlo_sc", bufs=2)
                    nc.scalar.mul(wlo_sc[:], wlo_ps[:], rden[:])
                    nc.vector.scalar_tensor_tensor(
                        out=out_sb[:, 0:D], in0=wsum_ps[:, 0:D], scalar=rden[:],
                        in1=wlo_sc[:], op0=Alu.mult, op1=Alu.add)
                    nc.sync.dma_start(out_dram[w], out_sb[:])
                    if _DEBUG:
                        dbg = sbuf_pool.tile([P, 2 * T_W], f32,
                                             name=f"dbg{rep}_{w}", tag="dbg",
                                             bufs=2)
                        nc.vector.tensor_copy(dbg[:, 0:T_W], st[w]["e_win"][:])
                        nc.vector.tensor_copy(dbg[:, T_W:], st[w]["z_win"][:])
                        nc.sync.dma_start(dbg_dram[w], dbg[:])

                # ---------------- emission schedule ----------------
                # Software-pipelined: e-matmuls lag their sigmoid batch by
                # E_LAG so the in-order PE stream never stalls on Act; z runs
                # in fine-grained chunks (ZB batches) so wsum pairs drain
                # incrementally; the small leftover tail of window 0 overlaps
                # window 1's anchor pass (anchor banks alternate by parity).
                E_LAG = _ELAG
                ZB = _ZB
                bd = list(range(0, NB, ZB)) + [NB]
                if bd[-2] == NB:
                    bd.pop()

                def pass2b(w):
                    from collections import deque
                    seq, ready = [], deque()

                    def drain(n):
                        for _ in range(n):
                            if ready:
                                seq.append(ready.popleft())

                    def after_e(eb):
                        seq.append(("e", w, eb))
                        drain(_DRAIN)
                        if (eb + 1) in bd[1:]:
                            c = bd.index(eb + 1) - 1
                            seq.append(("z", w, c))
                            for j in range(bd[c] * 2, bd[c + 1] * 2):
                                ready.append(("w", w, j))

                    for b in range(NB):
                        seq.append(("s", w, b))
                        drain(_DRAIN)
                        if b - E_LAG >= 0:
                            after_e(b - E_LAG)
                    for eb in range(max(0, NB - E_LAG), NB):
                        after_e(eb)
                    return seq, list(ready)

                def dispatch(seq):
                    for kind, w, i in seq:
                        if kind == "s":
                            emit_s_matmuls(w, i)
                        elif kind == "e":
                            emit_e_batch(w, i)
                        elif kind == "z":
                            emit_z_chunk(w, bd[i], bd[i + 1])
                        elif kind == "w":
                            emit_wsum_pair(w, i)
                        elif kind == "a":
                            emit_anchor_pair(w, i)

                emit_window_setup(0)
                emit_window_setup(1)
                for j in range(NP):
                    emit_anchor_pair(0, j)
                emit_fv(0)
                seq0, tail0 = pass2b(0)
                q1 = (len(seq0) * _Q1) // 8
                q2 = (len(seq0) * _Q2) // 8
                dispatch(seq0[:q1])
                dispatch(_weave2(seq0[q1:q2],
                                 [("a", 1, j) for j in range(NP)]))
                emit_fv(1)
                dispatch(seq0[q2:])
                dispatch(tail0)
                emit_output(0)
                seq1, tail1 = pass2b(1)
                dispatch(seq1)
                dispatch(tail1)
                emit_output(1)

            if loop_repeat is not None:
                import inspect as _insp
                _kw = {}
                if "staggered_reset" in _insp.signature(tc.For_i).parameters:
                    _kw["staggered_reset"] = _STAGGER
                with tc.For_i(0, loop_repeat, 1, **_kw):
                    for u in range(_UNROLL):
                        body(f"L{u}")
            else:
                for rep in range(repeat):
                    body(rep)

    return nc


def _weave2(a_items, b_items):
    """Proportionally interleave two lists, preserving each list's order."""
    na, nb = len(a_items), len(b_items)
    out = []
    ai = bi = 0
    while ai < na or bi < nb:
        if ai < na and (bi >= nb or ai * nb <= bi * na):
            out.append(a_items[ai]); ai += 1
        else:
            out.append(b_items[bi]); bi += 1
    return out


def _prepare(ifeat, Wu, Wv, bv, we, seg_ids):
    """Host-side shard + pad + layout. Returns (T_W, in_maps)."""
    ifeat = np.asarray(ifeat, dtype=np.float32)
    Wu = np.asarray(Wu, dtype=np.float32)
    Wv = np.asarray(Wv, dtype=np.float32)
    bv = np.asarray(bv, dtype=np.float32)
    we = np.asarray(we, dtype=np.float32)
    seg_ids = np.asarray(seg_ids)

    W = N_WINDOWS
    bounds = np.searchsorted(
        seg_ids, np.arange(0, B + 1, SEGS_PER_WINDOW), side="left")
    n_w = np.diff(bounds)
    T_W = max(4, int(-(-int(n_w.max()) // P)))
    T_W = ((T_W + 3) // 4) * 4
    NT = W_PER_CORE * T_W

    win = (seg_ids // SEGS_PER_WINDOW).astype(np.int64)
    pos = np.arange(N, dtype=np.int64) - bounds[win]
    sloc = (seg_ids % SEGS_PER_WINDOW).astype(np.int64)

    if8 = ifeat.astype(F8)
    # error-diffused fp8 rounding of nat: within each segment (nodes sorted),
    # carry the rounding residual forward so segment sums are nearly exact.
    natq = np.empty((N, D), dtype=F8)
    carry = np.zeros(D, dtype=np.float32)
    seg_np = np.asarray(seg_ids, dtype=np.int64)
    prev = -1
    for i in range(N):
        s = seg_np[i]
        if s != prev:
            carry[:] = 0.0
            prev = s
        v = ifeat[i] + carry
        q = v.astype(F8)
        carry = v - q.astype(np.float32)
        natq[i] = q
    natA = np.zeros((W, T_W * P, D), dtype=F8)
    natA[win, pos, :] = natq
    ifA = np.zeros((W, T_W * P, D), dtype=F8)   # nearest-rounded for fc_u
    ifA[win, pos, :] = if8
    ohwA = np.zeros((W, T_W * P, P), dtype=F8)
    ohwA[win, pos, sloc] = 1.0

    counts = np.bincount(np.asarray(seg_ids, dtype=np.int64), minlength=B)
    rcnt = (1.0 / np.maximum(counts, 1)).astype(np.float32).reshape(W, P, 1)

    wuT8 = (np.ascontiguousarray(Wu.T) * WU_SCALE).reshape(2, P, D)
    wuT8 = np.ascontiguousarray(wuT8.transpose(1, 0, 2)).astype(F8)  # [P,2,D]
    wvT8 = (np.ascontiguousarray(Wv.T) * WU_SCALE).reshape(2, P, D)
    wvT8 = np.ascontiguousarray(wvT8.transpose(1, 0, 2)).astype(F8)
    web = np.concatenate([we.reshape(2, P).T, -we.reshape(2, P).T],
                         axis=1).astype(BF)  # [dlo, (+h0,+h1,-h0,-h1)]
    bvb8 = np.tile(bv * FV_SCALE, (P, 1)).astype(np.float32)
    idb = np.eye(P, dtype=BF)
    one8 = np.ones((P, 2), dtype=F8)

    in_maps = []
    for c in range(N_CORES):
        X = natA[2 * c:2 * c + 2].reshape(W_PER_CORE, T_W, P, D)
        # nat [lane, (w,t), d]
        natp = np.ascontiguousarray(
            X.transpose(2, 0, 1, 3).reshape(P, NT, D))
        # ifT [d_lo, (w,t), kb, lane]
        Y = ifA[2 * c:2 * c + 2].reshape(W_PER_CORE, T_W, P, 2, P)
        iftp = np.ascontiguousarray(
            Y.transpose(4, 0, 1, 3, 2).reshape(P, NT, 2, P))
        O = ohwA[2 * c:2 * c + 2].reshape(W_PER_CORE, T_W, P, P)
        ohwp = np.ascontiguousarray(
            O.transpose(2, 0, 1, 3).reshape(P, NT, P))
        ohtp = np.ascontiguousarray(
            O.transpose(3, 0, 1, 2).reshape(P, NT, P))
        in_maps.append({
            "natp": natp, "iftp": iftp, "ohwp": ohwp, "ohtp": ohtp,
            "wuT8": wuT8, "wvT8": wvT8, "web": web, "bvb8": bvb8,
            "idb": idb, "one8": one8,
            "rcnt": rcnt[2 * c:2 * c + 2],
        })
    return T_W, in_maps


_DEBUG = False
_LAST = {}


def _run(ifeat, Wu, Wv, bv, we, seg_ids, trace=False):
    from concourse.bass_utils import run_bass_kernel_spmd

    T_W, in_maps = _prepare(ifeat, Wu, Wv, bv, we, seg_ids)
    nc = _build(T_W)
    _split_sync_waits(nc)
    res = run_bass_kernel_spmd(nc, in_maps, list(range(N_CORES)), trace=trace)
    _LAST["res"] = res
    _LAST["T_W"] = T_W
    _LAST["nc"] = nc
    _LAST["in_maps"] = in_maps

    out = np.empty((B, 2 * D), dtype=np.float32)
    for c in range(N_CORES):
        core_out = res.results[c]["out"]  # [W_PER_CORE, P, 2D]
        for wl in range(W_PER_CORE):
            w = c * W_PER_CORE + wl
            out[w * SEGS_PER_WINDOW:(w + 1) * SEGS_PER_WINDOW, :] = core_out[wl]
    return out


def kernel(ifeat, Wu, Wv, bv, we, seg_ids):
    return _run(ifeat, Wu, Wv, bv, we, seg_ids, trace=False)
